# revision 26
# baseline (speedup 1.0000x reference)
"""Causal self-attention (flipped mask: attend to k >= q) on 8 Trainium2 cores.

Sharding: 2-way data parallel over batch x 4-way head parallel (4 heads/core).
Each core computes x[b] -> qkv (its 4 heads) -> attention -> partial out-proj
(its 256 rows of Wo); the host sums the 4 partials per batch and adds bo.

Structure (v8, from v7's 193us):
  - Minimal PE prologue: warmup + exp-table preload + qk chains m=3,2 +
    v chains 12-15 only.  Everything else (qk m=1,0, v 0-11, all g=1
    qk chains, out-proj tiles) is emitted as paced FILLERS inside the
    ACT-bound attention j-loops (one unit per j, popped after the lag
    AV), so PE never idles while ACT streams exp.
  - DMA order: wq,wk,xT(m=3),wv,biases,xT2,msk,xT1,xT0,wo — matches
    first-use order of the new schedule; epool/warmup zeroing via DVE
    memsets instead of DMAs.
  - attention groups (g=0: n=3,2,1,0 then g=1: n=3,2,1,0): scores for a
    head pair land in one [128,1024] PSUM tile (row-group-concurrent K=64
    MM pair); one batched ACTIVATE Exp (bias -4 shift, softmax-invariant)
    per j; band blocks get narrowed exp APs + post-exp f16 mask-mul.
  - softmax denominators via ones-columns in the AV lhsT; recip/
    broadcast/muls deferred one group (DVE never waits on the DMA
    reshape round trip), EXCEPT at the very end where m11 runs as an
    in-loop filler of (0,g1) so d_tiles 4-7 can also be fillers there.
  - tail: only recip/muls of (0,g1) + d_tiles 0-3 remain, alternating
    between the psP and psS PSUM pools to double-buffer MM vs CAST evac.
Measured dead ends (v7): fp8 projections (rel err 4.5e-2), gpsimd
elementwise muls (2x slowdown), per-c0 split of the first xT DMA.
"""

import numpy as np

B, T, C = 2, 2048, 1024
H = 16
D = 64
NH = 4           # heads per core
HC = NH * D      # 256 local head cols
SCALE = 0.125    # 1/sqrt(D)
N_CORES = 8
ESHIFT = -4.0    # exp(s + ESHIFT): cancels in softmax, keeps e' in f16 range

NT = T // 128    # 16 t-tiles
NCC = C // 128   # 8 c-chunks
NQ = T // 512    # 4 q-chunks of 512
NJ = T // 128    # 16 kt-chunks of 128
EBUFS = 7

_CACHE = {}


def _build_nc():
    import concourse.tile as tile
    from concourse import bacc, mybir

    f32 = mybir.dt.float32
    f16 = mybir.dt.float16
    Exp = mybir.ActivationFunctionType.Exp
    Ident = mybir.ActivationFunctionType.Identity

    nc = bacc.Bacc(None, target_bir_lowering=False, debug=False)

    xT = nc.dram_tensor("xT", [C, T], f16, kind="ExternalInput")
    wq = nc.dram_tensor("wq", [C, HC], f16, kind="ExternalInput")
    wk = nc.dram_tensor("wk", [C, HC], f16, kind="ExternalInput")
    wv = nc.dram_tensor("wv", [C, HC], f16, kind="ExternalInput")
    bqs = nc.dram_tensor("bqs", [HC], f32, kind="ExternalInput")
    bk = nc.dram_tensor("bk", [HC], f32, kind="ExternalInput")
    bvb = nc.dram_tensor("bvb", [128, HC], f32, kind="ExternalInput")
    wo = nc.dram_tensor("wo", [HC, C], f16, kind="ExternalInput")
    mskM = nc.dram_tensor("mskM", [128, 4, 1024], f16, kind="ExternalInput")
    shf = nc.dram_tensor("shf", [128, 1], f32, kind="ExternalInput")
    out = nc.dram_tensor("out", [T, C], f16, kind="ExternalOutput")

    with tile.TileContext(nc) as tc, (
        tc.tile_pool(name="consts", bufs=1)) as consts, (
        tc.tile_pool(name="wts", bufs=1)) as wts, (
        tc.tile_pool(name="persist", bufs=1)) as persist:

        # ---- DMA order matters: matches first-use of the schedule ----
        wq_sb = wts.tile([128, NCC, HC], f16)
        nc.sync.dma_start(out=wq_sb, in_=wq.rearrange("(a p) n -> p a n", p=128))
        wk_sb = wts.tile([128, NCC, HC], f16)
        nc.sync.dma_start(out=wk_sb, in_=wk.rearrange("(a p) n -> p a n", p=128))

        wv_sb = wts.tile([128, NCC, HC], f16)
        nc.sync.dma_start(out=wv_sb, in_=wv.rearrange("(a p) n -> p a n", p=128))
        xT_sb = persist.tile([128, NCC, T], f16)
        nc.sync.dma_start(
            out=xT_sb[:, :, 1536:2048],
            in_=xT[:, 1536:2048].rearrange("(a p) q -> p a q", p=128),
        )
        bq_sb = consts.tile([128, 2], f32)
        nc.sync.dma_start(out=bq_sb, in_=bqs.rearrange("(a p) -> p a", p=128))
        bk_sb = consts.tile([128, 2], f32)
        nc.sync.dma_start(out=bk_sb, in_=bk.rearrange("(a p) -> p a", p=128))
        bvb_sb = consts.tile([128, NH, D], f32)
        nc.sync.dma_start(out=bvb_sb, in_=bvb.rearrange("p (h d) -> p h d", h=NH))
        shf_sb = consts.tile([128, 1], f32)
        nc.sync.dma_start(out=shf_sb, in_=shf[:, :])
        nc.sync.dma_start(
            out=xT_sb[:, :, 1024:1536],
            in_=xT[:, 1024:1536].rearrange("(a p) q -> p a q", p=128),
        )
        msk_sb = consts.tile([128, 4, 1024], f16)
        nc.sync.dma_start(out=msk_sb, in_=mskM[:, :, :])
        nc.sync.dma_start(
            out=xT_sb[:, :, 512:1024],
            in_=xT[:, 512:1024].rearrange("(a p) q -> p a q", p=128),
        )
        nc.sync.dma_start(
            out=xT_sb[:, :, 0:512],
            in_=xT[:, 0:512].rearrange("(a p) q -> p a q", p=128),
        )
        wo_sb = wts.tile([128, 2, C], f16)
        nc.sync.dma_start(out=wo_sb, in_=wo.rearrange("(a p) n -> p a n", p=128))

        # ---- persistent activations ----
        qT_sb = persist.tile([128, 2, T], f16)   # [2 head-pair chunks, T]
        kT_sb = persist.tile([128, 2, T], f16)
        # v, augmented: per t-tile, per pair g: [65 even | 130 odd]
        # even block: cols 0..63 = v(2g), col 64 = 1.0
        # odd block:  col 0 = 1.0 (offset 65), cols 64..127 = v(2g+1)
        v_sb = persist.tile([128, NT, 2, 195], f16)
        yT_sb = persist.tile([128, 2, T], f16)
        warm_sb = consts.tile([128, 1024], f16)
        ones_sb = consts.tile([128, 128], f16)
        tpre = consts.tile([128, 1], f32)

        # zero-init via DVE (no DMA traffic): warmup operand, v ones/pad
        nc.vector.memset(warm_sb, 0.0)
        nc.vector.memset(ones_sb, 1.0)
        nc.vector.memset(v_sb[:, :, :, 64:129], 0.0)
        nc.vector.memset(v_sb[:, :, :, 64:66], 1.0)

        def qk_mm(ps, g, m, c0, is_k):
            w_sb = wk_sb if is_k else wq_sb
            nc.tensor.matmul(
                ps,
                lhsT=(w_sb[:, c0, g * 128:(g + 1) * 128]),
                rhs=(xT_sb[:, c0, m * 512:(m + 1) * 512]),
                start=(c0 == 0), stop=(c0 == NCC - 1),
            )

        def qk_fin(ps, g, m, is_k):
            if is_k:
                nc.scalar.activation(
                    kT_sb[:, g, m * 512:(m + 1) * 512], ps, Ident,
                    bias=bk_sb[:, g:g + 1], scale=1.0,
                )
            else:
                nc.scalar.activation(
                    qT_sb[:, g, m * 512:(m + 1) * 512], ps, Ident,
                    bias=bq_sb[:, g:g + 1], scale=1.0,
                )

        def v_fin(ps, t0):
            psv4 = ps[:, 0:HC].rearrange("p (h d) -> p h d", h=NH)
            for gg in range(2):
                nc.vector.tensor_add(
                    v_sb[:, t0, gg, 0:64], psv4[:, 2 * gg, :],
                    bvb_sb[:, 2 * gg, :],
                )
                nc.vector.tensor_add(
                    v_sb[:, t0, gg, 129:193], psv4[:, 2 * gg + 1, :],
                    bvb_sb[:, 2 * gg + 1, :],
                )

        # ---- prologue: warmup + table preload + qk m=3,2 + v 12-15 ----
        with tc.tile_pool(name="psB", bufs=6, space="PSUM") as psB:
            with nc.named_scope("warmup"):
                for w in range(30):
                    pw = psB.tile([128, 512], f32, tag="pj")
                    nc.tensor.matmul(
                        pw, lhsT=warm_sb[:, 0:128], rhs=warm_sb[:, 0:512],
                        start=True, stop=True,
                    )
                # load the exp table set while PE warms up
                nc.scalar.activation(tpre, warm_sb[:, 0:1], Exp, scale=1.0)
            with nc.named_scope("phaseB0"):
                for i in (6, 7):   # m=3 qk chains + v 12-15
                    m, is_k = divmod(i, 2)
                    psqk = psB.tile([128, 512], f32, tag="pj")
                    psv0 = psB.tile([128, 512], f32, tag="pj")
                    psv1 = psB.tile([128, 512], f32, tag="pj")
                    t0a, t0b = 2 * i, 2 * i + 1
                    for c0 in range(NCC):
                        qk_mm(psqk, 0, m, c0, is_k)
                        nc.tensor.matmul(
                            psv0[:, 0:HC],
                            lhsT=(xT_sb[:, c0, t0a * 128:(t0a + 1) * 128]),
                            rhs=(wv_sb[:, c0, :]),
                            start=(c0 == 0), stop=(c0 == NCC - 1),
                        )
                        nc.tensor.matmul(
                            psv1[:, 0:HC],
                            lhsT=(xT_sb[:, c0, t0b * 128:(t0b + 1) * 128]),
                            rhs=(wv_sb[:, c0, :]),
                            start=(c0 == 0), stop=(c0 == NCC - 1),
                        )
                    qk_fin(psqk, 0, m, is_k)
                    v_fin(psv0, t0a)
                    v_fin(psv1, t0b)

        # ---- attention phases with everything else as in-loop fillers ----
        with (
            tc.tile_pool(name="epool", bufs=EBUFS) as epool,
            tc.tile_pool(name="rpool", bufs=3) as rpool,
            tc.tile_pool(name="opool", bufs=2) as opool,
            tc.tile_pool(name="psS", bufs=2, space="PSUM") as psS,
            tc.tile_pool(name="psY", bufs=1, space="PSUM") as psY,
            tc.tile_pool(name="psP", bufs=1, space="PSUM") as psP,
        ):
            # NaN guard: epool buffers are read through stale regions by the
            # band mask-mul before their first full write — zero them once
            # (DVE memsets run during the prologue, off the DMA path).
            for _ in range(EBUFS):
                et0 = epool.tile([128, 1024], f16, tag="e")
                nc.vector.memset(et0, 0.0)

            def qk_chain(g, m, is_k):
                def emit():
                    ps = psP.tile([128, 1024], f32, tag="p")
                    psh = ps[:, 0:512]
                    for c0 in range(NCC):
                        qk_mm(psh, g, m, c0, is_k)
                    if g == 0:
                        qk_fin(psh, g, m, is_k)
                    else:
                        dst = kT_sb if is_k else qT_sb
                        nc.vector.tensor_copy(
                            dst[:, 1, m * 512:(m + 1) * 512], psh)
                return emit

            def v_chain(t0):
                def emit():
                    ps = psP.tile([128, 1024], f32, tag="p")
                    for c0 in range(NCC):
                        nc.tensor.matmul(
                            ps[:, 0:HC],
                            lhsT=(xT_sb[:, c0, t0 * 128:(t0 + 1) * 128]),
                            rhs=(wv_sb[:, c0, :]),
                            start=(c0 == 0), stop=(c0 == NCC - 1),
                        )
                    v_fin(ps, t0)
                return emit

            def emit_group(n, g, fillers=None, fast_norm=False):
                fillers = list(fillers or [])
                qs = n * 512
                yt = psY.tile([128, 1024], f32, tag="y")
                ye = yt[:, 0:512]
                yo = yt[:, 512:1024]

                def emit_av(jj, e_t):
                    # bands 1,2: columns beyond the live region are exactly
                    # zero after the mask-mul AND the psY elements there were
                    # fully written by the (full-width) band-0 matmul, so
                    # narrowing is safe; band 0 stays full-width to clear
                    # the whole bank (start=True) with mask-zeroed e.
                    bav = jj - 4 * n
                    nbv = 512 if bav in (0, 3) or bav > 3 else 128 * (bav + 1)
                    nc.tensor.matmul(
                        ye[0:65, 0:nbv],
                        lhsT=(v_sb[:, jj, g, 0:65]),
                        rhs=(e_t[:, 0:nbv]),
                        start=(jj == 4 * n), stop=(jj == NJ - 1),
                    )
                    nc.tensor.matmul(
                        yo[:, 0:nbv],
                        lhsT=(v_sb[:, jj, g, 65:193]),
                        rhs=(e_t[:, 512:512 + nbv]),
                        start=(jj == 4 * n), stop=(jj == NJ - 1),
                    )

                lag = []
                for j in range(4 * n, NJ):
                    bnd = j - 4 * n
                    ks = j * 128
                    nb = min(512, 128 * (bnd + 1))
                    ps = psS.tile([128, 1024], f32, tag="s")
                    nc.tensor.matmul(
                        ps[:, 0:nb],
                        lhsT=(kT_sb[0:64, g, ks:ks + 128]),
                        rhs=(qT_sb[0:64, g, qs:qs + nb]),
                        start=True, stop=True,
                    )
                    nc.tensor.matmul(
                        ps[:, 512:512 + nb],
                        lhsT=(kT_sb[64:128, g, ks:ks + 128]),
                        rhs=(qT_sb[64:128, g, qs:qs + nb]),
                        start=True, stop=True,
                    )
                    if len(lag) >= 3:
                        emit_av(*lag.pop(0))
                    e_t = epool.tile([128, 1024], f16, tag="e")
                    e2 = e_t.rearrange("p (h q) -> p h q", h=2)
                    if bnd < 3:
                        p2 = ps.rearrange("p (h q) -> p h q", h=2)
                        nc.scalar.activation(
                            e2[:, :, 0:nb], p2[:, :, 0:nb], Exp,
                            bias=shf_sb[:, 0:1], scale=1.0,
                        )
                    else:
                        nc.scalar.activation(
                            e_t, ps, Exp, bias=shf_sb[:, 0:1], scale=1.0,
                        )
                    if bnd == 0 or bnd == 3:
                        # full-width: band 0 must zero stale cols for the
                        # full-width band-0 AV; band 3 has live cols to 512
                        nc.vector.tensor_mul(e_t, e_t, msk_sb[:, bnd, :])
                    elif bnd < 4:
                        m2 = msk_sb[:, bnd, :].rearrange("p (h q) -> p h q", h=2)
                        nc.vector.tensor_mul(
                            e2[:, :, 0:nb], e2[:, :, 0:nb], m2[:, :, 0:nb])
                    lag.append((j, e_t))
                    if fillers:
                        fillers.pop(0)()
                for item in lag:
                    emit_av(*item)
                for f in fillers:
                    f()
                # Evacuate unnormalized y (incl. denominator rows 64/0) to
                # SBUF f16 immediately — frees the psY buffer — and trigger
                # the denominator reshape DMAs ([1,512] -> [128,4]).  The
                # rest of the norm chain is returned as closures the caller
                # emits later so the in-order DVE queue never stalls on the
                # DMA round-trip.
                ysbE = rpool.tile([128, 512], f16, tag="ysbE")
                ysbO = rpool.tile([128, 512], f16, tag="ysbO")
                nc.vector.tensor_copy(ysbE, ye)
                nc.vector.tensor_copy(ysbO, yo)
                if not fast_norm:
                    rs = rpool.tile([128, 8], f16, tag="rs")
                    nc.sync.dma_start(out=rs[:, 0:4], in_=ysbE[64:65, :])
                    nc.sync.dma_start(out=rs[:, 4:8], in_=ysbO[0:1, :])

                state = {}

                def finish_recip():
                    if fast_norm:
                        # no DMA round trip: broadcast the raw denominator
                        # rows across partitions with a K=1 ones matmul (PE
                        # is idle here), then a 128-lane reciprocal — avoids
                        # both the DMA reshape latency and 1-lane reciprocals
                        pb = psY.tile([128, 1024], f32, tag="y")
                        nc.tensor.matmul(
                            pb[:, 0:512], lhsT=ones_sb[64:65, :],
                            rhs=ysbE[64:65, :], start=True, stop=True)
                        nc.tensor.matmul(
                            pb[:, 512:1024], lhsT=ones_sb[0:1, :],
                            rhs=ysbO[0:1, :], start=True, stop=True)
                        rr = rpool.tile([128, 1024], f16, tag="rt")
                        with nc.allow_low_precision(
                            reason="f16 softmax denominators; tol is 2e-2"
                        ):
                            nc.vector.reciprocal(rr[:, 0:512], pb[:, 0:512])
                            nc.vector.reciprocal(
                                rr[:, 512:1024], pb[:, 512:1024])
                        state["bsb"] = (rr[:, 0:512], rr[:, 512:1024])
                        return
                    rr = rpool.tile([128, 8], f16, tag="rr")
                    with nc.allow_low_precision(
                        reason="f16 softmax denominators; tol is 2e-2"
                    ):
                        nc.vector.reciprocal(rr, rs)
                    rt = rpool.tile([128, 1024], f16, tag="rt")
                    nc.sync.dma_start(out=rt[0:1, 0:512], in_=rr[:, 0:4])
                    nc.sync.dma_start(out=rt[0:1, 512:1024], in_=rr[:, 4:8])
                    bsbE = rpool.tile([128, 512], f16, tag="bsbE")
                    bsbO = rpool.tile([128, 512], f16, tag="bsbO")
                    nc.gpsimd.partition_broadcast(bsbE[:, :], rt[0:1, 0:512])
                    nc.gpsimd.partition_broadcast(bsbO[:, :], rt[0:1, 512:1024])
                    state["bsb"] = (bsbE, bsbO)

                def finish_muls():
                    bsbE, bsbO = state["bsb"]
                    nc.vector.tensor_mul(
                        yT_sb[0:64, g, qs:qs + 512], ysbE[0:64, :],
                        bsbE[0:64, :],
                    )
                    nc.vector.tensor_mul(
                        yT_sb[64:128, g, qs:qs + 512], ysbO[64:128, :],
                        bsbO[64:128, :],
                    )
                return finish_recip, finish_muls

            def d_tile(t0, pool=None, evac="dve"):
                def emit():
                    pd = (pool or psP).tile(
                        [128, 1024], f32, tag="p" if (pool or psP) is psP else "s")
                    for g2 in range(2):
                        nc.tensor.matmul(
                            pd[:, 0:512],
                            lhsT=(yT_sb[:, g2, t0 * 128:(t0 + 1) * 128]),
                            rhs=(wo_sb[:, g2, 0:512]),
                            start=(g2 == 0), stop=(g2 == 1),
                        )
                        nc.tensor.matmul(
                            pd[:, 512:1024],
                            lhsT=(yT_sb[:, g2, t0 * 128:(t0 + 1) * 128]),
                            rhs=(wo_sb[:, g2, 512:1024]),
                            start=(g2 == 0), stop=(g2 == 1),
                        )
                    o_sb = opool.tile([128, 1024], f16, tag="o")
                    if evac == "act":
                        # late d-tiles: DVE is backlogged with in-loop casts
                        # while ACT idles after the last exp — evacuate there
                        nc.scalar.copy(o_sb, pd)
                    else:
                        nc.vector.tensor_copy(o_sb, pd)
                    nc.sync.dma_start(
                        out=out[t0 * 128:(t0 + 1) * 128, :], in_=o_sb
                    )
                return emit

            with nc.named_scope("phaseCg0"):
                r30, m30 = emit_group(3, 0, fillers=[
                    qk_chain(0, 2, False), qk_chain(0, 2, True),
                    v_chain(8), v_chain(9), v_chain(10), v_chain(11)])
                r20, m20 = emit_group(2, 0, fillers=[
                    qk_chain(0, 1, False), qk_chain(0, 1, True),
                    v_chain(4), v_chain(5), v_chain(6), v_chain(7)])
                r30()
                r10, m10 = emit_group(1, 0, fillers=[
                    qk_chain(0, 0, False), qk_chain(0, 0, True),
                    v_chain(0), v_chain(1), v_chain(2), v_chain(3)])
                r20()
                m30()
                r00, m00 = emit_group(0, 0, fillers=[
                    qk_chain(1, 3, False), qk_chain(1, 3, True),
                    qk_chain(1, 2, False), qk_chain(1, 2, True)])
                r10()
                m20()
            with nc.named_scope("phaseCg1D"):
                r31, m31 = emit_group(3, 1, fillers=[
                    qk_chain(1, 0, False), qk_chain(1, 0, True)])
                r00()
                m10()
                r21, m21 = emit_group(2, 1, fillers=[
                    qk_chain(1, 1, False), qk_chain(1, 1, True)])
                m00()
                r31()
                m31()
                r11, m11 = emit_group(1, 1, fillers=[d_tile(12 + t) for t in range(4)])
                r21()
                m21()

                r01, m01 = emit_group(0, 1, fillers=(
                    [r11, m11] + [d_tile(4 + t) for t in range(4)]))
                # d8-11 run here, giving PE dense work that covers the
                # exposed norm-chain latency (rs DMA round trip + gpsimd
                # broadcasts) of the final group
                for t in range(4):
                    d_tile(8 + t, pool=(psS if t % 2 else psP), evac="act")()
                r01()
                m01()
                for t in range(4):
                    d_tile(t, pool=(psS if t % 2 else psP),
                           evac=("act" if t % 2 == 0 else "dve"))()

    nc.compile()
    return nc


def _host_consts():
    # multiplicative post-exp mask for the 4 band offsets b = j - 4n:
    # keep score[p, c] (kt-partition p, q-col c) iff c <= p + 128*b,
    # duplicated for the even/odd head halves of the [128,1024] e tile.
    p = np.arange(128)[:, None]
    c = np.arange(512)[None, :]
    blocks = []
    for b in range(4):
        m = (c <= p + 128 * b).astype(np.float16)
        blocks.append(np.concatenate([m, m], axis=1))
    mskM = np.stack(blocks, axis=1)  # [128, 4, 1024]
    shf = np.full((128, 1), ESHIFT, dtype=np.float32)
    return mskM, shf


def make_in_maps(x, Wqkv, bqkv, Wo, bo):
    x = np.asarray(x, dtype=np.float32)
    Wqkv = np.asarray(Wqkv, dtype=np.float32)
    bqkv = np.asarray(bqkv, dtype=np.float32)
    Wo = np.asarray(Wo, dtype=np.float32)
    mskM, shf = _host_consts()
    xT = [np.ascontiguousarray(x[b].T).astype(np.float16) for b in range(B)]
    in_maps = []
    for core in range(N_CORES):
        b, hg = divmod(core, 4)
        s = HC * hg
        in_maps.append({
            "xT": xT[b],
            "wq": np.ascontiguousarray(
                Wqkv[:, s:s + HC] * np.float32(SCALE)).astype(np.float16),
            "wk": np.ascontiguousarray(Wqkv[:, C + s:C + s + HC]).astype(np.float16),
            "wv": np.ascontiguousarray(Wqkv[:, 2 * C + s:2 * C + s + HC]).astype(np.float16),
            "bqs": np.ascontiguousarray(bqkv[s:s + HC]) * np.float32(SCALE),
            "bk": np.ascontiguousarray(bqkv[C + s:C + s + HC]),
            "bvb": np.ascontiguousarray(
                np.broadcast_to(bqkv[2 * C + s:2 * C + s + HC], (128, HC))
            ),
            "wo": np.ascontiguousarray(Wo[s:s + HC, :]).astype(np.float16),
            "mskM": mskM,
            "shf": shf,
        })
    return in_maps


def unshard(results, bo=None):
    out = np.empty((B, T, C), dtype=np.float32)
    for b in range(B):
        acc = results[4 * b]["out"].astype(np.float32)
        for hg in range(1, 4):
            acc = acc + results[4 * b + hg]["out"].astype(np.float32)
        out[b] = acc
    if bo is not None:
        out += np.asarray(bo, dtype=np.float32)[None, None, :]
    return out


def get_nc():
    if "nc" not in _CACHE:
        _CACHE["nc"] = _build_nc()
    return _CACHE["nc"]


def kernel(x, Wqkv, bqkv, Wo, bo):
    from concourse.bass_utils import run_bass_kernel_spmd

    nc = get_nc()
    in_maps = make_in_maps(x, Wqkv, bqkv, Wo, bo)
    res = run_bass_kernel_spmd(nc, in_maps, list(range(N_CORES)))
    return unshard(res.results, bo)


# revision 29
# speedup vs baseline: 1.0157x; 1.0157x over previous
"""Causal self-attention (flipped mask: attend to k >= q) on 8 Trainium2 cores.

Sharding: 2-way data parallel over batch x 4-way head parallel (4 heads/core).
Each core computes x[b] -> qkv (its 4 heads) -> attention -> partial out-proj
(its 256 rows of Wo); the host sums the 4 partials per batch and adds bo.

Structure (v12, 169us; v7 baseline was 193us):
  - Minimal PE prologue: 30 zero-matmul warmups (cover the ~19us
    HBM-bound input-DMA window — 8 cores x 7MB land simultaneously —
    and keep the HAM clock at 2.4GHz) + exp-table preload + qk chains
    m=3 + v chains 12-15.  Everything else (qk m=2,1,0, v 0-11, all g=1
    qk chains, most out-proj tiles) is emitted as paced FILLERS inside
    the ACT-bound attention j-loops (one unit per j, popped after the
    lag-2 AV), so PE never idles while ACT streams exp.
  - DMA order: wq,wk,wv,xT3,biases,shf,xT2,msk,xT1,xT0,wo — matches
    first-use order of the schedule; epool/warmup zeroing via DVE
    memsets instead of DMAs.
  - attention groups (g=0: n=3,2,1,0 then g=1: n=3,2,1,0): scores for a
    head pair land in one [128,1024] PSUM tile (row-group-concurrent K=64
    MM pair); one batched ACTIVATE Exp (bias -4 shift, softmax-invariant)
    per j; band tiles (bnd<3) narrow scores MMs, exp APs, mask-muls and
    AV MMs (bnd 1,2) to live columns only; bnd-0 AV stays full-width so
    start=True fully initializes the psY bank with mask-zeroed e.
  - softmax denominators via ones-columns in the AV lhsT; recip via the
    [1,512]->[128,4] DMA reshape (DVE reciprocal is ~6.5ns/elem/lane —
    a direct [128,512] recip costs 3.3us!), gpsimd partition_broadcast,
    all deferred one group so DVE never waits on the round trip.  At the
    very end, r11/m11 run as the FIRST in-loop fillers of (0,g1) (so
    d_tiles 4-7 can also be fillers there, with their PSUM-evac casts
    early in the DVE queue), and d_tiles 8-11 run between (0,g1) and
    r01/m01 with ACT (scalar.copy) evacuation — dense PE work covering
    the exposed final norm-chain latency while DVE stays clear.
  - tail: d_tiles 0-3 alternate psP/psS PSUM pools (double-buffer MM vs
    evac) and alternate ACT/DVE evacuation.
Measured dead ends: fp8 anywhere (rel err ~4.5e-2 vs 2e-2 tol), gpsimd
elementwise muls (2x slowdown), per-c0 split of the first xT DMA,
[128,512] or [1,512] DVE reciprocals (3.3us each), K=1 PE-matmul
denominator broadcast + wide recip (slower than the DMA reshape), AV
lag 3 / 7 e-bufs (+6us), splitting xT DMAs onto the ACT HWDGE ring
(front is HBM-bound, not ring-bound: ~1.4TB/s aggregate across cores).
"""

import numpy as np

B, T, C = 2, 2048, 1024
H = 16
D = 64
NH = 4           # heads per core
HC = NH * D      # 256 local head cols
SCALE = 0.125    # 1/sqrt(D)
N_CORES = 8
ESHIFT = -4.0    # exp(s + ESHIFT): cancels in softmax, keeps e' in f16 range

NT = T // 128    # 16 t-tiles
NCC = C // 128   # 8 c-chunks
NQ = T // 512    # 4 q-chunks of 512
NJ = T // 128    # 16 kt-chunks of 128
EBUFS = 6

_CACHE = {}


def _build_nc():
    import concourse.tile as tile
    from concourse import bacc, mybir

    f32 = mybir.dt.float32
    f16 = mybir.dt.float16
    Exp = mybir.ActivationFunctionType.Exp
    Ident = mybir.ActivationFunctionType.Identity

    nc = bacc.Bacc(None, target_bir_lowering=False, debug=False)

    xT = nc.dram_tensor("xT", [C, T], f16, kind="ExternalInput")
    wq = nc.dram_tensor("wq", [C, HC], f16, kind="ExternalInput")
    wk = nc.dram_tensor("wk", [C, HC], f16, kind="ExternalInput")
    wv = nc.dram_tensor("wv", [C, HC], f16, kind="ExternalInput")
    bqs = nc.dram_tensor("bqs", [HC], f32, kind="ExternalInput")
    bk = nc.dram_tensor("bk", [HC], f32, kind="ExternalInput")
    bvb = nc.dram_tensor("bvb", [128, HC], f32, kind="ExternalInput")
    wo = nc.dram_tensor("wo", [HC, C], f16, kind="ExternalInput")
    mskM = nc.dram_tensor("mskM", [128, 4, 1024], f16, kind="ExternalInput")
    shf = nc.dram_tensor("shf", [128, 1], f32, kind="ExternalInput")
    out = nc.dram_tensor("out", [T, C], f16, kind="ExternalOutput")

    with tile.TileContext(nc) as tc, (
        tc.tile_pool(name="consts", bufs=1)) as consts, (
        tc.tile_pool(name="wts", bufs=1)) as wts, (
        tc.tile_pool(name="persist", bufs=1)) as persist:

        # ---- DMA order matters: matches first-use of the schedule ----
        wq_sb = wts.tile([128, NCC, HC], f16)
        nc.sync.dma_start(out=wq_sb, in_=wq.rearrange("(a p) n -> p a n", p=128))
        wk_sb = wts.tile([128, NCC, HC], f16)
        nc.sync.dma_start(out=wk_sb, in_=wk.rearrange("(a p) n -> p a n", p=128))

        wv_sb = wts.tile([128, NCC, HC], f16)
        nc.sync.dma_start(out=wv_sb, in_=wv.rearrange("(a p) n -> p a n", p=128))
        xT_sb = persist.tile([128, NCC, T], f16)
        nc.sync.dma_start(
            out=xT_sb[:, :, 1536:2048],
            in_=xT[:, 1536:2048].rearrange("(a p) q -> p a q", p=128),
        )
        bq_sb = consts.tile([128, 2], f32)
        nc.sync.dma_start(out=bq_sb, in_=bqs.rearrange("(a p) -> p a", p=128))
        bk_sb = consts.tile([128, 2], f32)
        nc.sync.dma_start(out=bk_sb, in_=bk.rearrange("(a p) -> p a", p=128))
        bvb_sb = consts.tile([128, NH, D], f32)
        nc.sync.dma_start(out=bvb_sb, in_=bvb.rearrange("p (h d) -> p h d", h=NH))
        shf_sb = consts.tile([128, 1], f32)
        nc.sync.dma_start(out=shf_sb, in_=shf[:, :])
        nc.sync.dma_start(
            out=xT_sb[:, :, 1024:1536],
            in_=xT[:, 1024:1536].rearrange("(a p) q -> p a q", p=128),
        )
        msk_sb = consts.tile([128, 4, 1024], f16)
        nc.sync.dma_start(out=msk_sb, in_=mskM[:, :, :])
        nc.sync.dma_start(
            out=xT_sb[:, :, 512:1024],
            in_=xT[:, 512:1024].rearrange("(a p) q -> p a q", p=128),
        )
        nc.sync.dma_start(
            out=xT_sb[:, :, 0:512],
            in_=xT[:, 0:512].rearrange("(a p) q -> p a q", p=128),
        )
        wo_sb = wts.tile([128, 2, C], f16)
        nc.sync.dma_start(out=wo_sb, in_=wo.rearrange("(a p) n -> p a n", p=128))

        # ---- persistent activations ----
        qT_sb = persist.tile([128, 2, T], f16)   # [2 head-pair chunks, T]
        kT_sb = persist.tile([128, 2, T], f16)
        # v, augmented: per t-tile, per pair g: [65 even | 130 odd]
        # even block: cols 0..63 = v(2g), col 64 = 1.0
        # odd block:  col 0 = 1.0 (offset 65), cols 64..127 = v(2g+1)
        v_sb = persist.tile([128, NT, 2, 195], f16)
        yT_sb = persist.tile([128, 2, T], f16)
        warm_sb = consts.tile([128, 1024], f16)
        ones_sb = consts.tile([128, 128], f16)
        tpre = consts.tile([128, 1], f32)

        # zero-init via DVE (no DMA traffic): warmup operand, v ones/pad
        nc.vector.memset(warm_sb, 0.0)
        nc.vector.memset(ones_sb, 1.0)
        nc.vector.memset(v_sb[:, :, :, 64:129], 0.0)
        nc.vector.memset(v_sb[:, :, :, 64:66], 1.0)

        def qk_mm(ps, g, m, c0, is_k):
            w_sb = wk_sb if is_k else wq_sb
            nc.tensor.matmul(
                ps,
                lhsT=(w_sb[:, c0, g * 128:(g + 1) * 128]),
                rhs=(xT_sb[:, c0, m * 512:(m + 1) * 512]),
                start=(c0 == 0), stop=(c0 == NCC - 1),
            )

        def qk_fin(ps, g, m, is_k):
            if is_k:
                nc.scalar.activation(
                    kT_sb[:, g, m * 512:(m + 1) * 512], ps, Ident,
                    bias=bk_sb[:, g:g + 1], scale=1.0,
                )
            else:
                nc.scalar.activation(
                    qT_sb[:, g, m * 512:(m + 1) * 512], ps, Ident,
                    bias=bq_sb[:, g:g + 1], scale=1.0,
                )

        def v_fin(ps, t0):
            psv4 = ps[:, 0:HC].rearrange("p (h d) -> p h d", h=NH)
            for gg in range(2):
                nc.vector.tensor_add(
                    v_sb[:, t0, gg, 0:64], psv4[:, 2 * gg, :],
                    bvb_sb[:, 2 * gg, :],
                )
                nc.vector.tensor_add(
                    v_sb[:, t0, gg, 129:193], psv4[:, 2 * gg + 1, :],
                    bvb_sb[:, 2 * gg + 1, :],
                )

        # ---- prologue: warmup + table preload + qk m=3,2 + v 12-15 ----
        with tc.tile_pool(name="psB", bufs=6, space="PSUM") as psB:
            with nc.named_scope("warmup"):
                for w in range(30):
                    pw = psB.tile([128, 512], f32, tag="pj")
                    nc.tensor.matmul(
                        pw, lhsT=warm_sb[:, 0:128], rhs=warm_sb[:, 0:512],
                        start=True, stop=True,
                    )
                # load the exp table set while PE warms up
                nc.scalar.activation(tpre, warm_sb[:, 0:1], Exp, scale=1.0)
            with nc.named_scope("phaseB0"):
                for i in (6, 7):   # m=3 qk chains + v 12-15
                    m, is_k = divmod(i, 2)
                    psqk = psB.tile([128, 512], f32, tag="pj")
                    psv0 = psB.tile([128, 512], f32, tag="pj")
                    psv1 = psB.tile([128, 512], f32, tag="pj")
                    t0a, t0b = 2 * i, 2 * i + 1
                    for c0 in range(NCC):
                        qk_mm(psqk, 0, m, c0, is_k)
                        nc.tensor.matmul(
                            psv0[:, 0:HC],
                            lhsT=(xT_sb[:, c0, t0a * 128:(t0a + 1) * 128]),
                            rhs=(wv_sb[:, c0, :]),
                            start=(c0 == 0), stop=(c0 == NCC - 1),
                        )
                        nc.tensor.matmul(
                            psv1[:, 0:HC],
                            lhsT=(xT_sb[:, c0, t0b * 128:(t0b + 1) * 128]),
                            rhs=(wv_sb[:, c0, :]),
                            start=(c0 == 0), stop=(c0 == NCC - 1),
                        )
                    qk_fin(psqk, 0, m, is_k)
                    v_fin(psv0, t0a)
                    v_fin(psv1, t0b)

        # ---- attention phases with everything else as in-loop fillers ----
        with (
            tc.tile_pool(name="epool", bufs=EBUFS) as epool,
            tc.tile_pool(name="rpool", bufs=3) as rpool,
            tc.tile_pool(name="opool", bufs=2) as opool,
            tc.tile_pool(name="psS", bufs=2, space="PSUM") as psS,
            tc.tile_pool(name="psY", bufs=1, space="PSUM") as psY,
            tc.tile_pool(name="psP", bufs=1, space="PSUM") as psP,
        ):
            # NaN guard: epool buffers are read through stale regions by the
            # band mask-mul before their first full write — zero them once
            # (DVE memsets run during the prologue, off the DMA path).
            for _ in range(EBUFS):
                et0 = epool.tile([128, 1024], f16, tag="e")
                nc.vector.memset(et0, 0.0)

            def qk_chain(g, m, is_k):
                def emit():
                    ps = psP.tile([128, 1024], f32, tag="p")
                    psh = ps[:, 0:512]
                    for c0 in range(NCC):
                        qk_mm(psh, g, m, c0, is_k)
                    if g == 0:
                        qk_fin(psh, g, m, is_k)
                    else:
                        dst = kT_sb if is_k else qT_sb
                        nc.vector.tensor_copy(
                            dst[:, 1, m * 512:(m + 1) * 512], psh)
                return emit

            def v_chain(t0):
                def emit():
                    ps = psP.tile([128, 1024], f32, tag="p")
                    for c0 in range(NCC):
                        nc.tensor.matmul(
                            ps[:, 0:HC],
                            lhsT=(xT_sb[:, c0, t0 * 128:(t0 + 1) * 128]),
                            rhs=(wv_sb[:, c0, :]),
                            start=(c0 == 0), stop=(c0 == NCC - 1),
                        )
                    v_fin(ps, t0)
                return emit

            def emit_group(n, g, fillers=None, fast_norm=False):
                fillers = list(fillers or [])
                qs = n * 512
                yt = psY.tile([128, 1024], f32, tag="y")
                ye = yt[:, 0:512]
                yo = yt[:, 512:1024]

                def emit_av(jj, e_t):
                    # bands 1,2: columns beyond the live region are exactly
                    # zero after the mask-mul AND the psY elements there were
                    # fully written by the (full-width) band-0 matmul, so
                    # narrowing is safe; band 0 stays full-width to clear
                    # the whole bank (start=True) with mask-zeroed e.
                    bav = jj - 4 * n
                    nbv = 512 if bav in (0, 3) or bav > 3 else 128 * (bav + 1)
                    nc.tensor.matmul(
                        ye[0:65, 0:nbv],
                        lhsT=(v_sb[:, jj, g, 0:65]),
                        rhs=(e_t[:, 0:nbv]),
                        start=(jj == 4 * n), stop=(jj == NJ - 1),
                    )
                    nc.tensor.matmul(
                        yo[:, 0:nbv],
                        lhsT=(v_sb[:, jj, g, 65:193]),
                        rhs=(e_t[:, 512:512 + nbv]),
                        start=(jj == 4 * n), stop=(jj == NJ - 1),
                    )

                lag = []
                for j in range(4 * n, NJ):
                    bnd = j - 4 * n
                    ks = j * 128
                    nb = min(512, 128 * (bnd + 1))
                    ps = psS.tile([128, 1024], f32, tag="s")
                    nc.tensor.matmul(
                        ps[:, 0:nb],
                        lhsT=(kT_sb[0:64, g, ks:ks + 128]),
                        rhs=(qT_sb[0:64, g, qs:qs + nb]),
                        start=True, stop=True,
                    )
                    nc.tensor.matmul(
                        ps[:, 512:512 + nb],
                        lhsT=(kT_sb[64:128, g, ks:ks + 128]),
                        rhs=(qT_sb[64:128, g, qs:qs + nb]),
                        start=True, stop=True,
                    )
                    if len(lag) >= 2:
                        emit_av(*lag.pop(0))
                    e_t = epool.tile([128, 1024], f16, tag="e")
                    e2 = e_t.rearrange("p (h q) -> p h q", h=2)
                    if bnd < 3:
                        p2 = ps.rearrange("p (h q) -> p h q", h=2)
                        nc.scalar.activation(
                            e2[:, :, 0:nb], p2[:, :, 0:nb], Exp,
                            bias=shf_sb[:, 0:1], scale=1.0,
                        )
                    else:
                        nc.scalar.activation(
                            e_t, ps, Exp, bias=shf_sb[:, 0:1], scale=1.0,
                        )
                    if bnd == 0 or bnd == 3:
                        # full-width: band 0 must zero stale cols for the
                        # full-width band-0 AV; band 3 has live cols to 512
                        nc.vector.tensor_mul(e_t, e_t, msk_sb[:, bnd, :])
                    elif bnd < 4:
                        m2 = msk_sb[:, bnd, :].rearrange("p (h q) -> p h q", h=2)
                        nc.vector.tensor_mul(
                            e2[:, :, 0:nb], e2[:, :, 0:nb], m2[:, :, 0:nb])
                    lag.append((j, e_t))
                    if fillers:
                        fillers.pop(0)()
                for item in lag:
                    emit_av(*item)
                for f in fillers:
                    f()
                # Evacuate unnormalized y (incl. denominator rows 64/0) to
                # SBUF f16 immediately — frees the psY buffer — and trigger
                # the denominator reshape DMAs ([1,512] -> [128,4]).  The
                # rest of the norm chain is returned as closures the caller
                # emits later so the in-order DVE queue never stalls on the
                # DMA round-trip.
                ysbE = rpool.tile([128, 512], f16, tag="ysbE")
                ysbO = rpool.tile([128, 512], f16, tag="ysbO")
                nc.vector.tensor_copy(ysbE, ye)
                nc.vector.tensor_copy(ysbO, yo)
                if not fast_norm:
                    rs = rpool.tile([128, 8], f16, tag="rs")
                    nc.sync.dma_start(out=rs[:, 0:4], in_=ysbE[64:65, :])
                    nc.sync.dma_start(out=rs[:, 4:8], in_=ysbO[0:1, :])

                state = {}

                def finish_recip():
                    if fast_norm:
                        # no DMA round trip: broadcast the raw denominator
                        # rows across partitions with a K=1 ones matmul (PE
                        # is idle here), then a 128-lane reciprocal — avoids
                        # both the DMA reshape latency and 1-lane reciprocals
                        pb = psY.tile([128, 1024], f32, tag="y")
                        nc.tensor.matmul(
                            pb[:, 0:512], lhsT=ones_sb[64:65, :],
                            rhs=ysbE[64:65, :], start=True, stop=True)
                        nc.tensor.matmul(
                            pb[:, 512:1024], lhsT=ones_sb[0:1, :],
                            rhs=ysbO[0:1, :], start=True, stop=True)
                        rr = rpool.tile([128, 1024], f16, tag="rt")
                        with nc.allow_low_precision(
                            reason="f16 softmax denominators; tol is 2e-2"
                        ):
                            nc.vector.reciprocal(rr[:, 0:512], pb[:, 0:512])
                            nc.vector.reciprocal(
                                rr[:, 512:1024], pb[:, 512:1024])
                        state["bsb"] = (rr[:, 0:512], rr[:, 512:1024])
                        return
                    rr = rpool.tile([128, 8], f16, tag="rr")
                    with nc.allow_low_precision(
                        reason="f16 softmax denominators; tol is 2e-2"
                    ):
                        nc.vector.reciprocal(rr, rs)
                    rt = rpool.tile([128, 1024], f16, tag="rt")
                    nc.sync.dma_start(out=rt[0:1, 0:512], in_=rr[:, 0:4])
                    nc.sync.dma_start(out=rt[0:1, 512:1024], in_=rr[:, 4:8])
                    bsbE = rpool.tile([128, 512], f16, tag="bsbE")
                    bsbO = rpool.tile([128, 512], f16, tag="bsbO")
                    nc.gpsimd.partition_broadcast(bsbE[:, :], rt[0:1, 0:512])
                    nc.gpsimd.partition_broadcast(bsbO[:, :], rt[0:1, 512:1024])
                    state["bsb"] = (bsbE, bsbO)

                def finish_muls():
                    bsbE, bsbO = state["bsb"]
                    nc.vector.tensor_mul(
                        yT_sb[0:64, g, qs:qs + 512], ysbE[0:64, :],
                        bsbE[0:64, :],
                    )
                    nc.vector.tensor_mul(
                        yT_sb[64:128, g, qs:qs + 512], ysbO[64:128, :],
                        bsbO[64:128, :],
                    )
                return finish_recip, finish_muls

            def d_tile(t0, pool=None, evac="dve"):
                def emit():
                    pd = (pool or psP).tile(
                        [128, 1024], f32, tag="p" if (pool or psP) is psP else "s")
                    for g2 in range(2):
                        nc.tensor.matmul(
                            pd[:, 0:512],
                            lhsT=(yT_sb[:, g2, t0 * 128:(t0 + 1) * 128]),
                            rhs=(wo_sb[:, g2, 0:512]),
                            start=(g2 == 0), stop=(g2 == 1),
                        )
                        nc.tensor.matmul(
                            pd[:, 512:1024],
                            lhsT=(yT_sb[:, g2, t0 * 128:(t0 + 1) * 128]),
                            rhs=(wo_sb[:, g2, 512:1024]),
                            start=(g2 == 0), stop=(g2 == 1),
                        )
                    o_sb = opool.tile([128, 1024], f16, tag="o")
                    if evac == "act":
                        # late d-tiles: DVE is backlogged with in-loop casts
                        # while ACT idles after the last exp — evacuate there
                        nc.scalar.copy(o_sb, pd)
                    else:
                        nc.vector.tensor_copy(o_sb, pd)
                    nc.sync.dma_start(
                        out=out[t0 * 128:(t0 + 1) * 128, :], in_=o_sb
                    )
                return emit

            with nc.named_scope("phaseCg0"):
                r30, m30 = emit_group(3, 0, fillers=[
                    qk_chain(0, 2, False), qk_chain(0, 2, True),
                    v_chain(8), v_chain(9), v_chain(10), v_chain(11)])
                r20, m20 = emit_group(2, 0, fillers=[
                    qk_chain(0, 1, False), qk_chain(0, 1, True),
                    v_chain(4), v_chain(5), v_chain(6), v_chain(7)])
                r30()
                r10, m10 = emit_group(1, 0, fillers=[
                    qk_chain(0, 0, False), qk_chain(0, 0, True),
                    v_chain(0), v_chain(1), v_chain(2), v_chain(3)])
                r20()
                m30()
                r00, m00 = emit_group(0, 0, fillers=[
                    qk_chain(1, 3, False), qk_chain(1, 3, True),
                    qk_chain(1, 2, False), qk_chain(1, 2, True)])
                r10()
                m20()
            with nc.named_scope("phaseCg1D"):
                r31, m31 = emit_group(3, 1, fillers=[
                    qk_chain(1, 0, False), qk_chain(1, 0, True)])
                r00()
                m10()
                r21, m21 = emit_group(2, 1, fillers=[
                    qk_chain(1, 1, False), qk_chain(1, 1, True)])
                m00()
                r31()
                m31()
                r11, m11 = emit_group(1, 1, fillers=[d_tile(12 + t) for t in range(4)])
                r21()
                m21()

                r01, m01 = emit_group(0, 1, fillers=(
                    [r11, m11] + [d_tile(4 + t) for t in range(4)]))
                # d8-11 run here, giving PE dense work that covers the
                # exposed norm-chain latency (rs DMA round trip + gpsimd
                # broadcasts) of the final group
                for t in range(4):
                    d_tile(8 + t, pool=(psS if t % 2 else psP), evac="act")()
                r01()
                m01()
                for t in range(4):
                    d_tile(t, pool=(psS if t % 2 else psP),
                           evac=("act" if t % 2 == 0 else "dve"))()

    nc.compile()
    return nc


def _host_consts():
    # multiplicative post-exp mask for the 4 band offsets b = j - 4n:
    # keep score[p, c] (kt-partition p, q-col c) iff c <= p + 128*b,
    # duplicated for the even/odd head halves of the [128,1024] e tile.
    p = np.arange(128)[:, None]
    c = np.arange(512)[None, :]
    blocks = []
    for b in range(4):
        m = (c <= p + 128 * b).astype(np.float16)
        blocks.append(np.concatenate([m, m], axis=1))
    mskM = np.stack(blocks, axis=1)  # [128, 4, 1024]
    shf = np.full((128, 1), ESHIFT, dtype=np.float32)
    return mskM, shf


def make_in_maps(x, Wqkv, bqkv, Wo, bo):
    x = np.asarray(x, dtype=np.float32)
    Wqkv = np.asarray(Wqkv, dtype=np.float32)
    bqkv = np.asarray(bqkv, dtype=np.float32)
    Wo = np.asarray(Wo, dtype=np.float32)
    mskM, shf = _host_consts()
    xT = [np.ascontiguousarray(x[b].T).astype(np.float16) for b in range(B)]
    in_maps = []
    for core in range(N_CORES):
        b, hg = divmod(core, 4)
        s = HC * hg
        in_maps.append({
            "xT": xT[b],
            "wq": np.ascontiguousarray(
                Wqkv[:, s:s + HC] * np.float32(SCALE)).astype(np.float16),
            "wk": np.ascontiguousarray(Wqkv[:, C + s:C + s + HC]).astype(np.float16),
            "wv": np.ascontiguousarray(Wqkv[:, 2 * C + s:2 * C + s + HC]).astype(np.float16),
            "bqs": np.ascontiguousarray(bqkv[s:s + HC]) * np.float32(SCALE),
            "bk": np.ascontiguousarray(bqkv[C + s:C + s + HC]),
            "bvb": np.ascontiguousarray(
                np.broadcast_to(bqkv[2 * C + s:2 * C + s + HC], (128, HC))
            ),
            "wo": np.ascontiguousarray(Wo[s:s + HC, :]).astype(np.float16),
            "mskM": mskM,
            "shf": shf,
        })
    return in_maps


def unshard(results, bo=None):
    out = np.empty((B, T, C), dtype=np.float32)
    for b in range(B):
        acc = results[4 * b]["out"].astype(np.float32)
        for hg in range(1, 4):
            acc = acc + results[4 * b + hg]["out"].astype(np.float32)
        out[b] = acc
    if bo is not None:
        out += np.asarray(bo, dtype=np.float32)[None, None, :]
    return out


def get_nc():
    if "nc" not in _CACHE:
        _CACHE["nc"] = _build_nc()
    return _CACHE["nc"]


def kernel(x, Wqkv, bqkv, Wo, bo):
    from concourse.bass_utils import run_bass_kernel_spmd

    nc = get_nc()
    in_maps = make_in_maps(x, Wqkv, bqkv, Wo, bo)
    res = run_bass_kernel_spmd(nc, in_maps, list(range(N_CORES)))
    return unshard(res.results, bo)


# revision 33
# speedup vs baseline: 1.0643x; 1.0479x over previous
"""Causal self-attention (flipped mask: attend to k >= q) on 8 Trainium2 cores.

Sharding: 2-way data parallel over batch x 4-way head parallel (4 heads/core).
Each core computes x[b] -> qkv (its 4 heads) -> attention -> partial out-proj
(its 256 rows of Wo); the host sums the 4 partials per batch and adds bo.

Structure (v12, 169us; v7 baseline was 193us):
  - Minimal PE prologue: 30 zero-matmul warmups (cover the ~19us
    HBM-bound input-DMA window — 8 cores x 7MB land simultaneously —
    and keep the HAM clock at 2.4GHz) + exp-table preload + qk chains
    m=3 + v chains 12-15.  Everything else (qk m=2,1,0, v 0-11, all g=1
    qk chains, most out-proj tiles) is emitted as paced FILLERS inside
    the ACT-bound attention j-loops (one unit per j, popped after the
    lag-2 AV), so PE never idles while ACT streams exp.
  - DMA order: wq,wk,wv,xT3,biases,shf,xT2,msk,xT1,xT0,wo — matches
    first-use order of the schedule; epool/warmup zeroing via DVE
    memsets instead of DMAs.
  - attention groups (g=0: n=3,2,1,0 then g=1: n=3,2,1,0): scores for a
    head pair land in one [128,1024] PSUM tile (row-group-concurrent K=64
    MM pair); one batched ACTIVATE Exp (bias -4 shift, softmax-invariant)
    per j; band tiles (bnd<3) narrow scores MMs, exp APs, mask-muls and
    AV MMs (bnd 1,2) to live columns only; bnd-0 AV stays full-width so
    start=True fully initializes the psY bank with mask-zeroed e.
  - softmax denominators via ones-columns in the AV lhsT; recip via the
    [1,512]->[128,4] DMA reshape (DVE reciprocal is ~6.5ns/elem/lane —
    a direct [128,512] recip costs 3.3us!), gpsimd partition_broadcast,
    all deferred one group so DVE never waits on the round trip.  At the
    very end, r11/m11 run as the FIRST in-loop fillers of (0,g1) (so
    d_tiles 4-7 can also be fillers there, with their PSUM-evac casts
    early in the DVE queue), and d_tiles 8-11 run between (0,g1) and
    r01/m01 with ACT (scalar.copy) evacuation — dense PE work covering
    the exposed final norm-chain latency while DVE stays clear.
  - tail: d_tiles 0-3 alternate psP/psS PSUM pools (double-buffer MM vs
    evac) and alternate ACT/DVE evacuation.
Measured dead ends: fp8 anywhere (rel err ~4.5e-2 vs 2e-2 tol), gpsimd
elementwise muls (2x slowdown), per-c0 split of the first xT DMA,
[128,512] or [1,512] DVE reciprocals (3.3us each), K=1 PE-matmul
denominator broadcast + wide recip (slower than the DMA reshape), AV
lag 3 / 7 e-bufs (+6us), splitting xT DMAs onto the ACT HWDGE ring
(front is HBM-bound, not ring-bound: ~1.4TB/s aggregate across cores).
"""

import numpy as np

B, T, C = 2, 2048, 1024
H = 16
D = 64
NH = 4           # heads per core
HC = NH * D      # 256 local head cols
SCALE = 0.125    # 1/sqrt(D)
N_CORES = 8
ESHIFT = -4.0    # exp(s + ESHIFT): cancels in softmax, keeps e' in f16 range

NT = T // 128    # 16 t-tiles
NCC = C // 128   # 8 c-chunks
NQ = T // 512    # 4 q-chunks of 512
NJ = T // 128    # 16 kt-chunks of 128
EBUFS = 6

_CACHE = {}


def _build_nc():
    import concourse.tile as tile
    from concourse import bacc, mybir

    f32 = mybir.dt.float32
    f16 = mybir.dt.float16
    bf16 = mybir.dt.bfloat16
    Exp = mybir.ActivationFunctionType.Exp
    Ident = mybir.ActivationFunctionType.Identity

    nc = bacc.Bacc(None, target_bir_lowering=False, debug=False)

    xT = nc.dram_tensor("xT", [C, T], f16, kind="ExternalInput")
    wq = nc.dram_tensor("wq", [C, HC], f16, kind="ExternalInput")
    wk = nc.dram_tensor("wk", [C, HC], f16, kind="ExternalInput")
    wv = nc.dram_tensor("wv", [C, HC], f16, kind="ExternalInput")
    bqs = nc.dram_tensor("bqs", [HC], f32, kind="ExternalInput")
    bk = nc.dram_tensor("bk", [HC], f32, kind="ExternalInput")
    bvb = nc.dram_tensor("bvb", [128, HC], f32, kind="ExternalInput")
    wo = nc.dram_tensor("wo", [HC, C], f16, kind="ExternalInput")
    mskM = nc.dram_tensor("mskM", [128, 4, 1024], bf16, kind="ExternalInput")
    shf = nc.dram_tensor("shf", [128, 1], f32, kind="ExternalInput")
    out = nc.dram_tensor("out", [T, C], f16, kind="ExternalOutput")

    with tile.TileContext(nc) as tc, (
        tc.tile_pool(name="consts", bufs=1)) as consts, (
        tc.tile_pool(name="wts", bufs=1)) as wts, (
        tc.tile_pool(name="persist", bufs=1)) as persist:

        # ---- DMA order matters: matches first-use of the schedule ----
        wq_sb = wts.tile([128, NCC, HC], f16)
        nc.sync.dma_start(out=wq_sb, in_=wq.rearrange("(a p) n -> p a n", p=128))
        wk_sb = wts.tile([128, NCC, HC], f16)
        nc.sync.dma_start(out=wk_sb, in_=wk.rearrange("(a p) n -> p a n", p=128))

        wv_sb = wts.tile([128, NCC, HC], f16)
        nc.sync.dma_start(out=wv_sb, in_=wv.rearrange("(a p) n -> p a n", p=128))
        xT_sb = persist.tile([128, NCC, T], f16)
        nc.sync.dma_start(
            out=xT_sb[:, :, 1536:2048],
            in_=xT[:, 1536:2048].rearrange("(a p) q -> p a q", p=128),
        )
        bq_sb = consts.tile([128, 2], f32)
        nc.sync.dma_start(out=bq_sb, in_=bqs.rearrange("(a p) -> p a", p=128))
        bk_sb = consts.tile([128, 2], f32)
        nc.sync.dma_start(out=bk_sb, in_=bk.rearrange("(a p) -> p a", p=128))
        bvb_sb = consts.tile([128, NH, D], f32)
        nc.sync.dma_start(out=bvb_sb, in_=bvb.rearrange("p (h d) -> p h d", h=NH))
        shf_sb = consts.tile([128, 1], f32)
        nc.sync.dma_start(out=shf_sb, in_=shf[:, :])
        nc.sync.dma_start(
            out=xT_sb[:, :, 1024:1536],
            in_=xT[:, 1024:1536].rearrange("(a p) q -> p a q", p=128),
        )
        msk_sb = consts.tile([128, 4, 1024], bf16)
        nc.sync.dma_start(out=msk_sb, in_=mskM[:, :, :])
        nc.sync.dma_start(
            out=xT_sb[:, :, 512:1024],
            in_=xT[:, 512:1024].rearrange("(a p) q -> p a q", p=128),
        )
        nc.sync.dma_start(
            out=xT_sb[:, :, 0:512],
            in_=xT[:, 0:512].rearrange("(a p) q -> p a q", p=128),
        )
        wo_sb = wts.tile([128, 2, C], f16)
        nc.sync.dma_start(out=wo_sb, in_=wo.rearrange("(a p) n -> p a n", p=128))

        # ---- persistent activations ----
        qT_sb = persist.tile([128, 2, T], f16)   # [2 head-pair chunks, T]
        kT_sb = persist.tile([128, 2, T], f16)
        # v, augmented: per t-tile, per pair g: [65 even | 130 odd]
        # even block: cols 0..63 = v(2g), col 64 = 1.0
        # odd block:  col 0 = 1.0 (offset 65), cols 64..127 = v(2g+1)
        v_sb = persist.tile([128, NT, 2, 195], bf16)
        yT_sb = persist.tile([128, 2, T], f16)
        warm_sb = consts.tile([128, 1024], f16)
        ones_sb = consts.tile([128, 128], f16)
        tpre = consts.tile([128, 1], f32)

        # zero-init via DVE (no DMA traffic): warmup operand, v ones/pad
        nc.vector.memset(warm_sb, 0.0)
        nc.vector.memset(ones_sb, 1.0)
        nc.vector.memset(v_sb[:, :, :, 64:129], 0.0)
        nc.vector.memset(v_sb[:, :, :, 64:66], 1.0)

        def qk_mm(ps, g, m, c0, is_k):
            w_sb = wk_sb if is_k else wq_sb
            nc.tensor.matmul(
                ps,
                lhsT=(w_sb[:, c0, g * 128:(g + 1) * 128]),
                rhs=(xT_sb[:, c0, m * 512:(m + 1) * 512]),
                start=(c0 == 0), stop=(c0 == NCC - 1),
            )

        def qk_fin(ps, g, m, is_k):
            if is_k:
                nc.scalar.activation(
                    kT_sb[:, g, m * 512:(m + 1) * 512], ps, Ident,
                    bias=bk_sb[:, g:g + 1], scale=1.0,
                )
            else:
                nc.scalar.activation(
                    qT_sb[:, g, m * 512:(m + 1) * 512], ps, Ident,
                    bias=bq_sb[:, g:g + 1], scale=1.0,
                )

        def v_fin(ps, t0):
            psv4 = ps[:, 0:HC].rearrange("p (h d) -> p h d", h=NH)
            for gg in range(2):
                nc.vector.tensor_add(
                    v_sb[:, t0, gg, 0:64], psv4[:, 2 * gg, :],
                    bvb_sb[:, 2 * gg, :],
                )
                nc.vector.tensor_add(
                    v_sb[:, t0, gg, 129:193], psv4[:, 2 * gg + 1, :],
                    bvb_sb[:, 2 * gg + 1, :],
                )

        # ---- prologue: warmup + table preload + qk m=3,2 + v 12-15 ----
        with tc.tile_pool(name="psB", bufs=6, space="PSUM") as psB:
            with nc.named_scope("warmup"):
                for w in range(36):
                    pw = psB.tile([128, 512], f32, tag="pj")
                    nc.tensor.matmul(
                        pw, lhsT=warm_sb[:, 0:128], rhs=warm_sb[:, 0:512],
                        start=True, stop=True,
                    )
                # load the exp table set while PE warms up
                nc.scalar.activation(tpre, warm_sb[:, 0:1], Exp, scale=1.0)
            with nc.named_scope("phaseB0"):
                for i in (6, 7):   # m=3 qk chains + v 12-15
                    m, is_k = divmod(i, 2)
                    psqk = psB.tile([128, 512], f32, tag="pj")
                    psv0 = psB.tile([128, 512], f32, tag="pj")
                    psv1 = psB.tile([128, 512], f32, tag="pj")
                    t0a, t0b = 2 * i, 2 * i + 1
                    for c0 in range(NCC):
                        qk_mm(psqk, 0, m, c0, is_k)
                        nc.tensor.matmul(
                            psv0[:, 0:HC],
                            lhsT=(xT_sb[:, c0, t0a * 128:(t0a + 1) * 128]),
                            rhs=(wv_sb[:, c0, :]),
                            start=(c0 == 0), stop=(c0 == NCC - 1),
                        )
                        nc.tensor.matmul(
                            psv1[:, 0:HC],
                            lhsT=(xT_sb[:, c0, t0b * 128:(t0b + 1) * 128]),
                            rhs=(wv_sb[:, c0, :]),
                            start=(c0 == 0), stop=(c0 == NCC - 1),
                        )
                    qk_fin(psqk, 0, m, is_k)
                    v_fin(psv0, t0a)
                    v_fin(psv1, t0b)

        # ---- attention phases with everything else as in-loop fillers ----
        with (
            tc.tile_pool(name="epool", bufs=EBUFS) as epool,
            tc.tile_pool(name="rpool", bufs=3) as rpool,
            tc.tile_pool(name="opool", bufs=2) as opool,
            tc.tile_pool(name="psS", bufs=2, space="PSUM") as psS,
            tc.tile_pool(name="psY", bufs=1, space="PSUM") as psY,
            tc.tile_pool(name="psP", bufs=1, space="PSUM") as psP,
        ):
            # NaN guard: epool buffers are read through stale regions by the
            # band mask-mul before their first full write — zero them once
            # (DVE memsets run during the prologue, off the DMA path).
            for _ in range(EBUFS):
                et0 = epool.tile([128, 1024], bf16, tag="e")
                nc.vector.memset(et0, 0.0)

            _pt = {"i": 0}

            def ptile():
                # alternate between two half-bank slots so consecutive
                # filler chains overlap MMs with the previous chain's evac
                _pt["i"] ^= 1
                t = "pa" if _pt["i"] else "pb"
                return psP.tile([128, 512], f32, tag=t, name=t)

            def qk_chain(g, m, is_k):
                def emit():
                    psh = ptile()
                    for c0 in range(NCC):
                        qk_mm(psh, g, m, c0, is_k)
                    if g == 0:
                        qk_fin(psh, g, m, is_k)
                    else:
                        dst = kT_sb if is_k else qT_sb
                        nc.vector.tensor_copy(
                            dst[:, 1, m * 512:(m + 1) * 512], psh)
                return emit

            def v_chain(t0):
                def emit():
                    ps = ptile()
                    for c0 in range(NCC):
                        nc.tensor.matmul(
                            ps[:, 0:HC],
                            lhsT=(xT_sb[:, c0, t0 * 128:(t0 + 1) * 128]),
                            rhs=(wv_sb[:, c0, :]),
                            start=(c0 == 0), stop=(c0 == NCC - 1),
                        )
                    v_fin(ps, t0)
                return emit

            def emit_group(n, g, fillers=None, fast_norm=False):
                fillers = list(fillers or [])
                qs = n * 512
                yt = psY.tile([128, 1024], f32, tag="y")
                ye = yt[:, 0:512]
                yo = yt[:, 512:1024]

                def emit_av(jj, e_t):
                    # bands 1,2: columns beyond the live region are exactly
                    # zero after the mask-mul AND the psY elements there were
                    # fully written by the (full-width) band-0 matmul, so
                    # narrowing is safe; band 0 stays full-width to clear
                    # the whole bank (start=True) with mask-zeroed e.
                    bav = jj - 4 * n
                    nbv = 512 if bav in (0, 3) or bav > 3 else 128 * (bav + 1)
                    nc.tensor.matmul(
                        ye[0:65, 0:nbv],
                        lhsT=(v_sb[:, jj, g, 0:65]),
                        rhs=(e_t[:, 0:nbv]),
                        start=(jj == 4 * n), stop=(jj == NJ - 1),
                    )
                    nc.tensor.matmul(
                        yo[:, 0:nbv],
                        lhsT=(v_sb[:, jj, g, 65:193]),
                        rhs=(e_t[:, 512:512 + nbv]),
                        start=(jj == 4 * n), stop=(jj == NJ - 1),
                    )

                lag = []
                for j in range(4 * n, NJ):
                    bnd = j - 4 * n
                    ks = j * 128
                    nb = min(512, 128 * (bnd + 1))
                    ps = psS.tile([128, 1024], f32, tag="s")
                    nc.tensor.matmul(
                        ps[:, 0:nb],
                        lhsT=(kT_sb[0:64, g, ks:ks + 128]),
                        rhs=(qT_sb[0:64, g, qs:qs + nb]),
                        start=True, stop=True,
                    )
                    nc.tensor.matmul(
                        ps[:, 512:512 + nb],
                        lhsT=(kT_sb[64:128, g, ks:ks + 128]),
                        rhs=(qT_sb[64:128, g, qs:qs + nb]),
                        start=True, stop=True,
                    )
                    if len(lag) >= 2:
                        emit_av(*lag.pop(0))
                    e_t = epool.tile([128, 1024], bf16, tag="e")
                    e2 = e_t.rearrange("p (h q) -> p h q", h=2)
                    if bnd < 3:
                        p2 = ps.rearrange("p (h q) -> p h q", h=2)
                        nc.scalar.activation(
                            e2[:, :, 0:nb], p2[:, :, 0:nb], Exp,
                            bias=shf_sb[:, 0:1], scale=1.0,
                        )
                    else:
                        nc.scalar.activation(
                            e_t, ps, Exp, bias=shf_sb[:, 0:1], scale=1.0,
                        )
                    if bnd == 0 or bnd == 3:
                        # full-width: band 0 must zero stale cols for the
                        # full-width band-0 AV; band 3 has live cols to 512
                        nc.vector.tensor_mul(e_t, e_t, msk_sb[:, bnd, :])
                    elif bnd < 4:
                        m2 = msk_sb[:, bnd, :].rearrange("p (h q) -> p h q", h=2)
                        nc.vector.tensor_mul(
                            e2[:, :, 0:nb], e2[:, :, 0:nb], m2[:, :, 0:nb])
                    lag.append((j, e_t))
                    if fillers:
                        fillers.pop(0)()
                for item in lag:
                    emit_av(*item)
                for f in fillers:
                    f()
                # Evacuate unnormalized y (incl. denominator rows 64/0) to
                # SBUF f16 immediately — frees the psY buffer — and trigger
                # the denominator reshape DMAs ([1,512] -> [128,4]).  The
                # rest of the norm chain is returned as closures the caller
                # emits later so the in-order DVE queue never stalls on the
                # DMA round-trip.
                ysbE = rpool.tile([128, 512], f16, tag="ysbE")
                ysbO = rpool.tile([128, 512], f16, tag="ysbO")
                nc.vector.tensor_copy(ysbE, ye)
                nc.vector.tensor_copy(ysbO, yo)
                if not fast_norm:
                    rs = rpool.tile([128, 8], f16, tag="rs")
                    nc.sync.dma_start(out=rs[:, 0:4], in_=ysbE[64:65, :])
                    nc.sync.dma_start(out=rs[:, 4:8], in_=ysbO[0:1, :])

                state = {}

                def finish_recip():
                    if fast_norm:
                        # no DMA round trip: broadcast the raw denominator
                        # rows across partitions with a K=1 ones matmul (PE
                        # is idle here), then a 128-lane reciprocal — avoids
                        # both the DMA reshape latency and 1-lane reciprocals
                        pb = psY.tile([128, 1024], f32, tag="y")
                        nc.tensor.matmul(
                            pb[:, 0:512], lhsT=ones_sb[64:65, :],
                            rhs=ysbE[64:65, :], start=True, stop=True)
                        nc.tensor.matmul(
                            pb[:, 512:1024], lhsT=ones_sb[0:1, :],
                            rhs=ysbO[0:1, :], start=True, stop=True)
                        rr = rpool.tile([128, 1024], f16, tag="rt")
                        with nc.allow_low_precision(
                            reason="f16 softmax denominators; tol is 2e-2"
                        ):
                            nc.vector.reciprocal(rr[:, 0:512], pb[:, 0:512])
                            nc.vector.reciprocal(
                                rr[:, 512:1024], pb[:, 512:1024])
                        state["bsb"] = (rr[:, 0:512], rr[:, 512:1024])
                        return
                    rr = rpool.tile([128, 8], f16, tag="rr")
                    with nc.allow_low_precision(
                        reason="f16 softmax denominators; tol is 2e-2"
                    ):
                        nc.vector.reciprocal(rr, rs)
                    rt = rpool.tile([128, 1024], f16, tag="rt")
                    nc.sync.dma_start(out=rt[0:1, 0:512], in_=rr[:, 0:4])
                    nc.sync.dma_start(out=rt[0:1, 512:1024], in_=rr[:, 4:8])
                    bsbE = rpool.tile([128, 512], f16, tag="bsbE")
                    bsbO = rpool.tile([128, 512], f16, tag="bsbO")
                    nc.gpsimd.partition_broadcast(bsbE[:, :], rt[0:1, 0:512])
                    nc.gpsimd.partition_broadcast(bsbO[:, :], rt[0:1, 512:1024])
                    state["bsb"] = (bsbE, bsbO)

                def finish_muls():
                    bsbE, bsbO = state["bsb"]
                    nc.vector.tensor_mul(
                        yT_sb[0:64, g, qs:qs + 512], ysbE[0:64, :],
                        bsbE[0:64, :],
                    )
                    nc.vector.tensor_mul(
                        yT_sb[64:128, g, qs:qs + 512], ysbO[64:128, :],
                        bsbO[64:128, :],
                    )
                return finish_recip, finish_muls

            def d_tile(t0, pool=None, evac="dve"):
                def emit():
                    # column-major halves: evacuate each 512-col half right
                    # after its accumulation so the slot frees sooner and
                    # the next chain's MMs overlap this one's evac
                    o_sb = opool.tile([128, 1024], f16, tag="o")
                    for h in range(2):
                        if pool is None or pool is psP:
                            pd = ptile()
                        else:
                            pds = pool.tile(
                                [128, 1024], f32, tag="s", name="pds")
                            pd = pds[:, 512 * h:512 * h + 512]
                        for g2 in range(2):
                            nc.tensor.matmul(
                                pd,
                                lhsT=(yT_sb[:, g2, t0 * 128:(t0 + 1) * 128]),
                                rhs=(wo_sb[:, g2, 512 * h:512 * h + 512]),
                                start=(g2 == 0), stop=(g2 == 1),
                            )
                        if evac == "act":
                            # DVE backlogged with in-loop casts while ACT
                            # idles after the last exp — evacuate there
                            nc.scalar.copy(o_sb[:, 512 * h:512 * h + 512], pd)
                        else:
                            nc.vector.tensor_copy(
                                o_sb[:, 512 * h:512 * h + 512], pd)
                    nc.sync.dma_start(
                        out=out[t0 * 128:(t0 + 1) * 128, :], in_=o_sb
                    )
                return emit

            with nc.named_scope("phaseCg0"):
                r30, m30 = emit_group(3, 0, fillers=[
                    qk_chain(0, 2, False), qk_chain(0, 2, True),
                    v_chain(8), v_chain(9), v_chain(10), v_chain(11)])
                r20, m20 = emit_group(2, 0, fillers=[
                    qk_chain(0, 1, False), qk_chain(0, 1, True),
                    v_chain(4), v_chain(5), v_chain(6), v_chain(7)])
                r30()
                r10, m10 = emit_group(1, 0, fillers=[
                    qk_chain(0, 0, False), qk_chain(0, 0, True),
                    v_chain(0), v_chain(1), v_chain(2), v_chain(3)])
                r20()
                m30()
                r00, m00 = emit_group(0, 0, fillers=[
                    qk_chain(1, 3, False), qk_chain(1, 3, True),
                    qk_chain(1, 2, False), qk_chain(1, 2, True)])
                r10()
                m20()
            with nc.named_scope("phaseCg1D"):
                r31, m31 = emit_group(3, 1, fillers=[
                    qk_chain(1, 0, False), qk_chain(1, 0, True)])
                r00()
                m10()
                r21, m21 = emit_group(2, 1, fillers=[
                    qk_chain(1, 1, False), qk_chain(1, 1, True)])
                m00()
                r31()
                m31()
                r11, m11 = emit_group(1, 1, fillers=[d_tile(12 + t) for t in range(4)])
                r21()
                m21()

                r01, m01 = emit_group(0, 1, fillers=(
                    [r11, m11] + [d_tile(4 + t) for t in range(4)]))
                # d8-11 run here, giving PE dense work that covers the
                # exposed norm-chain latency (rs DMA round trip + gpsimd
                # broadcasts) of the final group
                for t in range(4):
                    d_tile(8 + t, pool=(psS if t % 2 else psP), evac="act")()
                r01()
                m01()
                for t in range(4):
                    d_tile(t, pool=(psS if t % 2 else psP),
                           evac=("dve" if t % 2 == 0 else "act"))()

    nc.compile()
    return nc


def _host_consts():
    # multiplicative post-exp mask for the 4 band offsets b = j - 4n:
    # keep score[p, c] (kt-partition p, q-col c) iff c <= p + 128*b,
    # duplicated for the even/odd head halves of the [128,1024] e tile.
    p = np.arange(128)[:, None]
    c = np.arange(512)[None, :]
    blocks = []
    for b in range(4):
        m = (c <= p + 128 * b)
        blocks.append(np.concatenate([m, m], axis=1))
    import ml_dtypes
    mskM = np.stack(blocks, axis=1).astype(ml_dtypes.bfloat16)  # [128, 4, 1024]
    shf = np.full((128, 1), ESHIFT, dtype=np.float32)
    return mskM, shf


def make_in_maps(x, Wqkv, bqkv, Wo, bo):
    x = np.asarray(x, dtype=np.float32)
    Wqkv = np.asarray(Wqkv, dtype=np.float32)
    bqkv = np.asarray(bqkv, dtype=np.float32)
    Wo = np.asarray(Wo, dtype=np.float32)
    mskM, shf = _host_consts()
    xT = [np.ascontiguousarray(x[b].T).astype(np.float16) for b in range(B)]
    in_maps = []
    for core in range(N_CORES):
        b, hg = divmod(core, 4)
        s = HC * hg
        in_maps.append({
            "xT": xT[b],
            "wq": np.ascontiguousarray(
                Wqkv[:, s:s + HC] * np.float32(SCALE)).astype(np.float16),
            "wk": np.ascontiguousarray(Wqkv[:, C + s:C + s + HC]).astype(np.float16),
            "wv": np.ascontiguousarray(Wqkv[:, 2 * C + s:2 * C + s + HC]).astype(np.float16),
            "bqs": np.ascontiguousarray(bqkv[s:s + HC]) * np.float32(SCALE),
            "bk": np.ascontiguousarray(bqkv[C + s:C + s + HC]),
            "bvb": np.ascontiguousarray(
                np.broadcast_to(bqkv[2 * C + s:2 * C + s + HC], (128, HC))
            ),
            "wo": np.ascontiguousarray(Wo[s:s + HC, :]).astype(np.float16),
            "mskM": mskM,
            "shf": shf,
        })
    return in_maps


def unshard(results, bo=None):
    out = np.empty((B, T, C), dtype=np.float32)
    for b in range(B):
        acc = results[4 * b]["out"].astype(np.float32)
        for hg in range(1, 4):
            acc = acc + results[4 * b + hg]["out"].astype(np.float32)
        out[b] = acc
    if bo is not None:
        out += np.asarray(bo, dtype=np.float32)[None, None, :]
    return out


def get_nc():
    if "nc" not in _CACHE:
        _CACHE["nc"] = _build_nc()
    return _CACHE["nc"]


def kernel(x, Wqkv, bqkv, Wo, bo):
    from concourse.bass_utils import run_bass_kernel_spmd

    nc = get_nc()
    in_maps = make_in_maps(x, Wqkv, bqkv, Wo, bo)
    res = run_bass_kernel_spmd(nc, in_maps, list(range(N_CORES)))
    return unshard(res.results, bo)


# revision 34
# speedup vs baseline: 1.0799x; 1.0146x over previous
"""Causal self-attention (flipped mask: attend to k >= q) on 8 Trainium2 cores.

Sharding: 2-way data parallel over batch x 4-way head parallel (4 heads/core).
Each core computes x[b] -> qkv (its 4 heads) -> attention -> partial out-proj
(its 256 rows of Wo); the host sums the 4 partials per batch and adds bo.

Structure (v14, 164us; v7 baseline was 193us):
  - Minimal PE prologue: 36 zero-matmul warmups (cover the ~19us
    HBM-bound input-DMA window — 8 cores x 7MB land simultaneously —
    and keep the HAM clock at 2.4GHz) + exp-table preload + qk chains
    m=3 + v chains 12-15.  Everything else (qk m=2,1,0, v 0-11, all g=1
    qk chains, most out-proj tiles) is emitted as paced FILLERS inside
    the ACT-bound attention j-loops (one unit per j, popped after the
    lag-2 AV), so PE never idles while ACT streams exp.
  - DMA order: wq,wk,wv,xT3,biases,shf,xT2,msk,xT1,xT0,wo — matches
    first-use order of the schedule; epool/warmup zeroing via DVE
    memsets instead of DMAs.
  - attention groups (g=0: n=3,2,1,0 then g=1: n=3,2,1,0): scores for a
    head pair land in one [128,1024] PSUM tile (row-group-concurrent K=64
    MM pair); one batched ACTIVATE Exp (bias -4 shift, softmax-invariant)
    per j; band tiles (bnd<3) narrow scores MMs, exp APs, mask-muls and
    AV MMs (bnd 1,2) to live columns only; bnd-0 AV stays full-width so
    start=True fully initializes the psY bank with mask-zeroed e.
  - softmax denominators via ones-columns in the AV lhsT; recip via the
    [1,512]->[128,4] DMA reshape (DVE reciprocal is ~6.5ns/elem/lane —
    a direct [128,512] recip costs 3.3us!), gpsimd partition_broadcast,
    all deferred one group so DVE never waits on the round trip.  At the
    very end, r11/m11 run as the FIRST in-loop fillers of (0,g1) (so
    d_tiles 4-7 can also be fillers there, with their PSUM-evac casts
    early in the DVE queue), and d_tiles 8-11 run between (0,g1) and
    r01/m01 with ACT (scalar.copy) evacuation — dense PE work covering
    the exposed final norm-chain latency while DVE stays clear.
  - tail: d_tiles 0-3 alternate psP/psS PSUM pools (double-buffer MM vs
    evac) and alternate DVE/ACT evacuation.
  - v14: e tiles / v_sb / mask are BFLOAT16 (f16 tensor_tensor runs at
    1x on DVE; bf16 gets the 2x packed mode — halves the mask-muls) at
    rel-err cost 7.5e-4 -> 2.1e-3, still 10x inside the 2e-2 tol; all
    psP filler chains alternate two half-bank slots (pa/pb) and d_tiles
    evacuate each 512-col half right after its accumulation, so
    consecutive chains overlap MM with evac instead of serializing on
    the single psP slot.
Measured dead ends: fp8 anywhere (rel err ~4.5e-2 vs 2e-2 tol), gpsimd
elementwise muls (2x slowdown), per-c0 split of the first xT DMA,
[128,512] or [1,512] DVE reciprocals (3.3us each), K=1 PE-matmul
denominator broadcast + wide recip (slower than the DMA reshape), AV
lag 3 / 7 e-bufs (+6us), splitting xT DMAs onto the ACT HWDGE ring
(front is HBM-bound, not ring-bound: ~1.4TB/s aggregate across cores).
"""

import numpy as np

B, T, C = 2, 2048, 1024
H = 16
D = 64
NH = 4           # heads per core
HC = NH * D      # 256 local head cols
SCALE = 0.125    # 1/sqrt(D)
N_CORES = 8
ESHIFT = -4.0    # exp(s + ESHIFT): cancels in softmax, keeps e' in f16 range

NT = T // 128    # 16 t-tiles
NCC = C // 128   # 8 c-chunks
NQ = T // 512    # 4 q-chunks of 512
NJ = T // 128    # 16 kt-chunks of 128
EBUFS = 6

_CACHE = {}


def _build_nc():
    import concourse.tile as tile
    from concourse import bacc, mybir

    f32 = mybir.dt.float32
    f16 = mybir.dt.float16
    bf16 = mybir.dt.bfloat16
    Exp = mybir.ActivationFunctionType.Exp
    Ident = mybir.ActivationFunctionType.Identity

    nc = bacc.Bacc(None, target_bir_lowering=False, debug=False)

    xT = nc.dram_tensor("xT", [C, T], f16, kind="ExternalInput")
    wq = nc.dram_tensor("wq", [C, HC], f16, kind="ExternalInput")
    wk = nc.dram_tensor("wk", [C, HC], f16, kind="ExternalInput")
    wv = nc.dram_tensor("wv", [C, HC], f16, kind="ExternalInput")
    bqs = nc.dram_tensor("bqs", [HC], f32, kind="ExternalInput")
    bk = nc.dram_tensor("bk", [HC], f32, kind="ExternalInput")
    bvb = nc.dram_tensor("bvb", [128, HC], f32, kind="ExternalInput")
    wo = nc.dram_tensor("wo", [HC, C], f16, kind="ExternalInput")
    mskM = nc.dram_tensor("mskM", [128, 4, 1024], bf16, kind="ExternalInput")
    shf = nc.dram_tensor("shf", [128, 1], f32, kind="ExternalInput")
    out = nc.dram_tensor("out", [T, C], f16, kind="ExternalOutput")

    with tile.TileContext(nc) as tc, (
        tc.tile_pool(name="consts", bufs=1)) as consts, (
        tc.tile_pool(name="wts", bufs=1)) as wts, (
        tc.tile_pool(name="persist", bufs=1)) as persist:

        # ---- DMA order matters: matches first-use of the schedule ----
        wq_sb = wts.tile([128, NCC, HC], f16)
        nc.sync.dma_start(out=wq_sb, in_=wq.rearrange("(a p) n -> p a n", p=128))
        wk_sb = wts.tile([128, NCC, HC], f16)
        nc.sync.dma_start(out=wk_sb, in_=wk.rearrange("(a p) n -> p a n", p=128))

        wv_sb = wts.tile([128, NCC, HC], f16)
        nc.sync.dma_start(out=wv_sb, in_=wv.rearrange("(a p) n -> p a n", p=128))
        xT_sb = persist.tile([128, NCC, T], f16)
        nc.sync.dma_start(
            out=xT_sb[:, :, 1536:2048],
            in_=xT[:, 1536:2048].rearrange("(a p) q -> p a q", p=128),
        )
        bq_sb = consts.tile([128, 2], f32)
        nc.sync.dma_start(out=bq_sb, in_=bqs.rearrange("(a p) -> p a", p=128))
        bk_sb = consts.tile([128, 2], f32)
        nc.sync.dma_start(out=bk_sb, in_=bk.rearrange("(a p) -> p a", p=128))
        bvb_sb = consts.tile([128, NH, D], f32)
        nc.sync.dma_start(out=bvb_sb, in_=bvb.rearrange("p (h d) -> p h d", h=NH))
        shf_sb = consts.tile([128, 1], f32)
        nc.sync.dma_start(out=shf_sb, in_=shf[:, :])
        nc.sync.dma_start(
            out=xT_sb[:, :, 1024:1536],
            in_=xT[:, 1024:1536].rearrange("(a p) q -> p a q", p=128),
        )
        msk_sb = consts.tile([128, 4, 1024], bf16)
        nc.sync.dma_start(out=msk_sb, in_=mskM[:, :, :])
        nc.sync.dma_start(
            out=xT_sb[:, :, 512:1024],
            in_=xT[:, 512:1024].rearrange("(a p) q -> p a q", p=128),
        )
        nc.sync.dma_start(
            out=xT_sb[:, :, 0:512],
            in_=xT[:, 0:512].rearrange("(a p) q -> p a q", p=128),
        )
        wo_sb = wts.tile([128, 2, C], f16)
        nc.sync.dma_start(out=wo_sb, in_=wo.rearrange("(a p) n -> p a n", p=128))

        # ---- persistent activations ----
        qT_sb = persist.tile([128, 2, T], f16)   # [2 head-pair chunks, T]
        kT_sb = persist.tile([128, 2, T], f16)
        # v, augmented: per t-tile, per pair g: [65 even | 130 odd]
        # even block: cols 0..63 = v(2g), col 64 = 1.0
        # odd block:  col 0 = 1.0 (offset 65), cols 64..127 = v(2g+1)
        v_sb = persist.tile([128, NT, 2, 195], bf16)
        yT_sb = persist.tile([128, 2, T], f16)
        warm_sb = consts.tile([128, 1024], f16)
        ones_sb = consts.tile([128, 128], f16)
        tpre = consts.tile([128, 1], f32)

        # zero-init via DVE (no DMA traffic): warmup operand, v ones/pad
        nc.vector.memset(warm_sb, 0.0)
        nc.vector.memset(ones_sb, 1.0)
        nc.vector.memset(v_sb[:, :, :, 64:129], 0.0)
        nc.vector.memset(v_sb[:, :, :, 64:66], 1.0)

        def qk_mm(ps, g, m, c0, is_k):
            w_sb = wk_sb if is_k else wq_sb
            nc.tensor.matmul(
                ps,
                lhsT=(w_sb[:, c0, g * 128:(g + 1) * 128]),
                rhs=(xT_sb[:, c0, m * 512:(m + 1) * 512]),
                start=(c0 == 0), stop=(c0 == NCC - 1),
            )

        def qk_fin(ps, g, m, is_k):
            if is_k:
                nc.scalar.activation(
                    kT_sb[:, g, m * 512:(m + 1) * 512], ps, Ident,
                    bias=bk_sb[:, g:g + 1], scale=1.0,
                )
            else:
                nc.scalar.activation(
                    qT_sb[:, g, m * 512:(m + 1) * 512], ps, Ident,
                    bias=bq_sb[:, g:g + 1], scale=1.0,
                )

        def v_fin(ps, t0):
            psv4 = ps[:, 0:HC].rearrange("p (h d) -> p h d", h=NH)
            for gg in range(2):
                nc.vector.tensor_add(
                    v_sb[:, t0, gg, 0:64], psv4[:, 2 * gg, :],
                    bvb_sb[:, 2 * gg, :],
                )
                nc.vector.tensor_add(
                    v_sb[:, t0, gg, 129:193], psv4[:, 2 * gg + 1, :],
                    bvb_sb[:, 2 * gg + 1, :],
                )

        # ---- prologue: warmup + table preload + qk m=3,2 + v 12-15 ----
        with tc.tile_pool(name="psB", bufs=6, space="PSUM") as psB:
            with nc.named_scope("warmup"):
                for w in range(36):
                    pw = psB.tile([128, 512], f32, tag="pj")
                    nc.tensor.matmul(
                        pw, lhsT=warm_sb[:, 0:128], rhs=warm_sb[:, 0:512],
                        start=True, stop=True,
                    )
                # load the exp table set while PE warms up
                nc.scalar.activation(tpre, warm_sb[:, 0:1], Exp, scale=1.0)
            with nc.named_scope("phaseB0"):
                for i in (6, 7):   # m=3 qk chains + v 12-15
                    m, is_k = divmod(i, 2)
                    psqk = psB.tile([128, 512], f32, tag="pj")
                    psv0 = psB.tile([128, 512], f32, tag="pj")
                    psv1 = psB.tile([128, 512], f32, tag="pj")
                    t0a, t0b = 2 * i, 2 * i + 1
                    for c0 in range(NCC):
                        qk_mm(psqk, 0, m, c0, is_k)
                        nc.tensor.matmul(
                            psv0[:, 0:HC],
                            lhsT=(xT_sb[:, c0, t0a * 128:(t0a + 1) * 128]),
                            rhs=(wv_sb[:, c0, :]),
                            start=(c0 == 0), stop=(c0 == NCC - 1),
                        )
                        nc.tensor.matmul(
                            psv1[:, 0:HC],
                            lhsT=(xT_sb[:, c0, t0b * 128:(t0b + 1) * 128]),
                            rhs=(wv_sb[:, c0, :]),
                            start=(c0 == 0), stop=(c0 == NCC - 1),
                        )
                    qk_fin(psqk, 0, m, is_k)
                    v_fin(psv0, t0a)
                    v_fin(psv1, t0b)

        # ---- attention phases with everything else as in-loop fillers ----
        with (
            tc.tile_pool(name="epool", bufs=EBUFS) as epool,
            tc.tile_pool(name="rpool", bufs=3) as rpool,
            tc.tile_pool(name="opool", bufs=2) as opool,
            tc.tile_pool(name="psS", bufs=2, space="PSUM") as psS,
            tc.tile_pool(name="psY", bufs=1, space="PSUM") as psY,
            tc.tile_pool(name="psP", bufs=1, space="PSUM") as psP,
        ):
            # NaN guard: epool buffers are read through stale regions by the
            # band mask-mul before their first full write — zero them once
            # (DVE memsets run during the prologue, off the DMA path).
            for _ in range(EBUFS):
                et0 = epool.tile([128, 1024], bf16, tag="e")
                nc.vector.memset(et0, 0.0)

            _pt = {"i": 0}

            def ptile():
                # alternate between two half-bank slots so consecutive
                # filler chains overlap MMs with the previous chain's evac
                _pt["i"] ^= 1
                t = "pa" if _pt["i"] else "pb"
                return psP.tile([128, 512], f32, tag=t, name=t)

            def qk_chain(g, m, is_k):
                def emit():
                    psh = ptile()
                    for c0 in range(NCC):
                        qk_mm(psh, g, m, c0, is_k)
                    if g == 0:
                        qk_fin(psh, g, m, is_k)
                    else:
                        dst = kT_sb if is_k else qT_sb
                        nc.vector.tensor_copy(
                            dst[:, 1, m * 512:(m + 1) * 512], psh)
                return emit

            def v_chain(t0):
                def emit():
                    ps = ptile()
                    for c0 in range(NCC):
                        nc.tensor.matmul(
                            ps[:, 0:HC],
                            lhsT=(xT_sb[:, c0, t0 * 128:(t0 + 1) * 128]),
                            rhs=(wv_sb[:, c0, :]),
                            start=(c0 == 0), stop=(c0 == NCC - 1),
                        )
                    v_fin(ps, t0)
                return emit

            def emit_group(n, g, fillers=None, fast_norm=False):
                fillers = list(fillers or [])
                qs = n * 512
                yt = psY.tile([128, 1024], f32, tag="y")
                ye = yt[:, 0:512]
                yo = yt[:, 512:1024]

                def emit_av(jj, e_t):
                    # bands 1,2: columns beyond the live region are exactly
                    # zero after the mask-mul AND the psY elements there were
                    # fully written by the (full-width) band-0 matmul, so
                    # narrowing is safe; band 0 stays full-width to clear
                    # the whole bank (start=True) with mask-zeroed e.
                    bav = jj - 4 * n
                    nbv = 512 if bav in (0, 3) or bav > 3 else 128 * (bav + 1)
                    nc.tensor.matmul(
                        ye[0:65, 0:nbv],
                        lhsT=(v_sb[:, jj, g, 0:65]),
                        rhs=(e_t[:, 0:nbv]),
                        start=(jj == 4 * n), stop=(jj == NJ - 1),
                    )
                    nc.tensor.matmul(
                        yo[:, 0:nbv],
                        lhsT=(v_sb[:, jj, g, 65:193]),
                        rhs=(e_t[:, 512:512 + nbv]),
                        start=(jj == 4 * n), stop=(jj == NJ - 1),
                    )

                lag = []
                for j in range(4 * n, NJ):
                    bnd = j - 4 * n
                    ks = j * 128
                    nb = min(512, 128 * (bnd + 1))
                    ps = psS.tile([128, 1024], f32, tag="s")
                    nc.tensor.matmul(
                        ps[:, 0:nb],
                        lhsT=(kT_sb[0:64, g, ks:ks + 128]),
                        rhs=(qT_sb[0:64, g, qs:qs + nb]),
                        start=True, stop=True,
                    )
                    nc.tensor.matmul(
                        ps[:, 512:512 + nb],
                        lhsT=(kT_sb[64:128, g, ks:ks + 128]),
                        rhs=(qT_sb[64:128, g, qs:qs + nb]),
                        start=True, stop=True,
                    )
                    if len(lag) >= 2:
                        emit_av(*lag.pop(0))
                    e_t = epool.tile([128, 1024], bf16, tag="e")
                    e2 = e_t.rearrange("p (h q) -> p h q", h=2)
                    if bnd < 3:
                        p2 = ps.rearrange("p (h q) -> p h q", h=2)
                        nc.scalar.activation(
                            e2[:, :, 0:nb], p2[:, :, 0:nb], Exp,
                            bias=shf_sb[:, 0:1], scale=1.0,
                        )
                    else:
                        nc.scalar.activation(
                            e_t, ps, Exp, bias=shf_sb[:, 0:1], scale=1.0,
                        )
                    if bnd == 0 or bnd == 3:
                        # full-width: band 0 must zero stale cols for the
                        # full-width band-0 AV; band 3 has live cols to 512
                        nc.vector.tensor_mul(e_t, e_t, msk_sb[:, bnd, :])
                    elif bnd < 4:
                        m2 = msk_sb[:, bnd, :].rearrange("p (h q) -> p h q", h=2)
                        nc.vector.tensor_mul(
                            e2[:, :, 0:nb], e2[:, :, 0:nb], m2[:, :, 0:nb])
                    lag.append((j, e_t))
                    if fillers:
                        fillers.pop(0)()
                for item in lag:
                    emit_av(*item)
                for f in fillers:
                    f()
                # Evacuate unnormalized y (incl. denominator rows 64/0) to
                # SBUF f16 immediately — frees the psY buffer — and trigger
                # the denominator reshape DMAs ([1,512] -> [128,4]).  The
                # rest of the norm chain is returned as closures the caller
                # emits later so the in-order DVE queue never stalls on the
                # DMA round-trip.
                ysbE = rpool.tile([128, 512], f16, tag="ysbE")
                ysbO = rpool.tile([128, 512], f16, tag="ysbO")
                nc.vector.tensor_copy(ysbE, ye)
                nc.vector.tensor_copy(ysbO, yo)
                if not fast_norm:
                    rs = rpool.tile([128, 8], f16, tag="rs")
                    nc.sync.dma_start(out=rs[:, 0:4], in_=ysbE[64:65, :])
                    nc.sync.dma_start(out=rs[:, 4:8], in_=ysbO[0:1, :])

                state = {}

                def finish_recip():
                    if fast_norm:
                        # no DMA round trip: broadcast the raw denominator
                        # rows across partitions with a K=1 ones matmul (PE
                        # is idle here), then a 128-lane reciprocal — avoids
                        # both the DMA reshape latency and 1-lane reciprocals
                        pb = psY.tile([128, 1024], f32, tag="y")
                        nc.tensor.matmul(
                            pb[:, 0:512], lhsT=ones_sb[64:65, :],
                            rhs=ysbE[64:65, :], start=True, stop=True)
                        nc.tensor.matmul(
                            pb[:, 512:1024], lhsT=ones_sb[0:1, :],
                            rhs=ysbO[0:1, :], start=True, stop=True)
                        rr = rpool.tile([128, 1024], f16, tag="rt")
                        with nc.allow_low_precision(
                            reason="f16 softmax denominators; tol is 2e-2"
                        ):
                            nc.vector.reciprocal(rr[:, 0:512], pb[:, 0:512])
                            nc.vector.reciprocal(
                                rr[:, 512:1024], pb[:, 512:1024])
                        state["bsb"] = (rr[:, 0:512], rr[:, 512:1024])
                        return
                    rr = rpool.tile([128, 8], f16, tag="rr")
                    with nc.allow_low_precision(
                        reason="f16 softmax denominators; tol is 2e-2"
                    ):
                        nc.vector.reciprocal(rr, rs)
                    rt = rpool.tile([128, 1024], f16, tag="rt")
                    nc.sync.dma_start(out=rt[0:1, 0:512], in_=rr[:, 0:4])
                    nc.sync.dma_start(out=rt[0:1, 512:1024], in_=rr[:, 4:8])
                    bsbE = rpool.tile([128, 512], f16, tag="bsbE")
                    bsbO = rpool.tile([128, 512], f16, tag="bsbO")
                    nc.gpsimd.partition_broadcast(bsbE[:, :], rt[0:1, 0:512])
                    nc.gpsimd.partition_broadcast(bsbO[:, :], rt[0:1, 512:1024])
                    state["bsb"] = (bsbE, bsbO)

                def finish_muls():
                    bsbE, bsbO = state["bsb"]
                    nc.vector.tensor_mul(
                        yT_sb[0:64, g, qs:qs + 512], ysbE[0:64, :],
                        bsbE[0:64, :],
                    )
                    nc.vector.tensor_mul(
                        yT_sb[64:128, g, qs:qs + 512], ysbO[64:128, :],
                        bsbO[64:128, :],
                    )
                return finish_recip, finish_muls

            def d_tile(t0, pool=None, evac="dve"):
                def emit():
                    # column-major halves: evacuate each 512-col half right
                    # after its accumulation so the slot frees sooner and
                    # the next chain's MMs overlap this one's evac
                    o_sb = opool.tile([128, 1024], f16, tag="o")
                    for h in range(2):
                        if pool is None or pool is psP:
                            pd = ptile()
                        else:
                            pds = pool.tile(
                                [128, 1024], f32, tag="s", name="pds")
                            pd = pds[:, 512 * h:512 * h + 512]
                        for g2 in range(2):
                            nc.tensor.matmul(
                                pd,
                                lhsT=(yT_sb[:, g2, t0 * 128:(t0 + 1) * 128]),
                                rhs=(wo_sb[:, g2, 512 * h:512 * h + 512]),
                                start=(g2 == 0), stop=(g2 == 1),
                            )
                        if evac == "act":
                            # DVE backlogged with in-loop casts while ACT
                            # idles after the last exp — evacuate there
                            nc.scalar.copy(o_sb[:, 512 * h:512 * h + 512], pd)
                        else:
                            nc.vector.tensor_copy(
                                o_sb[:, 512 * h:512 * h + 512], pd)
                    nc.sync.dma_start(
                        out=out[t0 * 128:(t0 + 1) * 128, :], in_=o_sb
                    )
                return emit

            with nc.named_scope("phaseCg0"):
                r30, m30 = emit_group(3, 0, fillers=[
                    qk_chain(0, 2, False), qk_chain(0, 2, True),
                    v_chain(8), v_chain(9), v_chain(10), v_chain(11)])
                r20, m20 = emit_group(2, 0, fillers=[
                    qk_chain(0, 1, False), qk_chain(0, 1, True),
                    v_chain(4), v_chain(5), v_chain(6), v_chain(7)])
                r30()
                r10, m10 = emit_group(1, 0, fillers=[
                    qk_chain(0, 0, False), qk_chain(0, 0, True),
                    v_chain(0), v_chain(1), v_chain(2), v_chain(3)])
                r20()
                m30()
                r00, m00 = emit_group(0, 0, fillers=[
                    qk_chain(1, 3, False), qk_chain(1, 3, True),
                    qk_chain(1, 2, False), qk_chain(1, 2, True)])
                r10()
                m20()
            with nc.named_scope("phaseCg1D"):
                r31, m31 = emit_group(3, 1, fillers=[
                    qk_chain(1, 0, False), qk_chain(1, 0, True)])
                r00()
                m10()
                r21, m21 = emit_group(2, 1, fillers=[
                    qk_chain(1, 1, False), qk_chain(1, 1, True)])
                m00()
                r31()
                m31()
                r11, m11 = emit_group(1, 1, fillers=[d_tile(12 + t) for t in range(4)])
                r21()
                m21()

                r01, m01 = emit_group(0, 1, fillers=(
                    [r11, m11] + [d_tile(4 + t) for t in range(4)]))
                # d8-11 run here, giving PE dense work that covers the
                # exposed norm-chain latency (rs DMA round trip + gpsimd
                # broadcasts) of the final group
                for t in range(4):
                    d_tile(8 + t, pool=(psS if t % 2 else psP), evac="act")()
                r01()
                m01()
                for t in range(4):
                    d_tile(t, pool=(psS if t % 2 else psP),
                           evac=("dve" if t % 2 == 0 else "act"))()

    nc.compile()
    return nc


def _host_consts():
    # multiplicative post-exp mask for the 4 band offsets b = j - 4n:
    # keep score[p, c] (kt-partition p, q-col c) iff c <= p + 128*b,
    # duplicated for the even/odd head halves of the [128,1024] e tile.
    p = np.arange(128)[:, None]
    c = np.arange(512)[None, :]
    blocks = []
    for b in range(4):
        m = (c <= p + 128 * b)
        blocks.append(np.concatenate([m, m], axis=1))
    import ml_dtypes
    mskM = np.stack(blocks, axis=1).astype(ml_dtypes.bfloat16)  # [128, 4, 1024]
    shf = np.full((128, 1), ESHIFT, dtype=np.float32)
    return mskM, shf


def make_in_maps(x, Wqkv, bqkv, Wo, bo):
    x = np.asarray(x, dtype=np.float32)
    Wqkv = np.asarray(Wqkv, dtype=np.float32)
    bqkv = np.asarray(bqkv, dtype=np.float32)
    Wo = np.asarray(Wo, dtype=np.float32)
    mskM, shf = _host_consts()
    xT = [np.ascontiguousarray(x[b].T).astype(np.float16) for b in range(B)]
    in_maps = []
    for core in range(N_CORES):
        b, hg = divmod(core, 4)
        s = HC * hg
        in_maps.append({
            "xT": xT[b],
            "wq": np.ascontiguousarray(
                Wqkv[:, s:s + HC] * np.float32(SCALE)).astype(np.float16),
            "wk": np.ascontiguousarray(Wqkv[:, C + s:C + s + HC]).astype(np.float16),
            "wv": np.ascontiguousarray(Wqkv[:, 2 * C + s:2 * C + s + HC]).astype(np.float16),
            "bqs": np.ascontiguousarray(bqkv[s:s + HC]) * np.float32(SCALE),
            "bk": np.ascontiguousarray(bqkv[C + s:C + s + HC]),
            "bvb": np.ascontiguousarray(
                np.broadcast_to(bqkv[2 * C + s:2 * C + s + HC], (128, HC))
            ),
            "wo": np.ascontiguousarray(Wo[s:s + HC, :]).astype(np.float16),
            "mskM": mskM,
            "shf": shf,
        })
    return in_maps


def unshard(results, bo=None):
    out = np.empty((B, T, C), dtype=np.float32)
    for b in range(B):
        acc = results[4 * b]["out"].astype(np.float32)
        for hg in range(1, 4):
            acc = acc + results[4 * b + hg]["out"].astype(np.float32)
        out[b] = acc
    if bo is not None:
        out += np.asarray(bo, dtype=np.float32)[None, None, :]
    return out


def get_nc():
    if "nc" not in _CACHE:
        _CACHE["nc"] = _build_nc()
    return _CACHE["nc"]


def kernel(x, Wqkv, bqkv, Wo, bo):
    from concourse.bass_utils import run_bass_kernel_spmd

    nc = get_nc()
    in_maps = make_in_maps(x, Wqkv, bqkv, Wo, bo)
    res = run_bass_kernel_spmd(nc, in_maps, list(range(N_CORES)))
    return unshard(res.results, bo)
